# revision 1
# baseline (speedup 1.0000x reference)
"""Bipolar morphological conv2d kernel for Trainium2 (8 NeuronCores).

Math: reference computes, per output position and out-channel c,
    y = m(lp1,K1) - m(lp1,K2) - m(lp2,K1) + m(lp2,K2) + bias
with m(logp, k)[c] = exp(max_p(logp_p + k_pc)), lp1 = log(max(patch, .1)),
lp2 = log(max(-patch, .1)).

Since exp is monotone, exp(max_p(log(max(x,.1)) + k)) = max_p(max(x,.1)*K_pc)
with K = exp(k) > 0.  Further, the clamp folds into a per-channel constant:
    max_p(max(x_p,.1)*K_pc) = max(U_c, max_p(x_p*K_pc)),  U_c = .1*max_p K_pc
(because entries with x_p <= .1 contribute x_p*K <= .1*K <= U_c, and the true
value is always >= U_c).  Likewise the "-x" side is max(U_c, max_p(-x_p*K_pc)).
So the whole op is 4 max-times reductions over unclamped products x_p*K_pc.

Device strategy (data-parallel, one batch image per core):
  - partitions = 128 = [64 out-channels "A side" (+x) | 64 out-channels "B side" (-x)]
  - free dim   = 900 output positions, addressed as [30 rows, 30 cols] windows
    (row stride 32) into the pixel-linear broadcast row
  - x row per input channel is broadcast across partitions as [+x;...;-x;...]
    via a K=1 PE matmul (lhsT = [+1]*64+[-1]*64) into PSUM, staged to SBUF by
    the Scalar engine.
  - per (tap, ci) one fused scalar_tensor_tensor (mult then max) per kernel
    accumulator: acc_k = max(acc_k, xwin * K_k[(tap,ci), c])  -- 576 DVE ops,
    which is the roofline: DVE f32 3-src ops run at 1 elem/cycle/lane.
  - combine: one accumulating PE matmul pair per 128-position chunk computes
    (accA1-accB1)-(accA2-accB2) transposed to position-major; add bias; DMA.
Host precomputes exp(k), U_c, the packed per-partition scalar tables, and the
transposed/padded x rows.
"""

import os
from contextlib import ExitStack

import numpy as np

import concourse.bass as bass
import concourse.mybir as mybir
from concourse import bacc
import concourse.tile as tile
from concourse.bass_utils import run_bass_kernel_spmd

N_CORES = 8
H = W = C = 32
COUT = 64
HO = WO = 30
NPIX = H * W          # 1024
FD = HO * WO          # 900 output positions, accessed as [30, 30] windows
XLEN = 1026           # broadcast-row length: max tap offset 66 + 30*32 window
P = 288               # 3*3*32 patch size

F32 = mybir.dt.float32
F16 = mybir.dt.float16
_cache: dict = {}
last_results = None


def _ensure_axon_ntff_hook():
    """The trimmed agent image lacks antenv.axon_hooks; recreate it so
    run_bass_kernel_spmd(trace=True) can capture NTFF profiles. No-op on
    failure (tracing then just degrades)."""
    import sys
    import types

    try:
        import antenv.axon_hooks  # noqa: F401
        return
    except ImportError:
        pass
    try:
        mod = types.ModuleType("antenv.axon_hooks")
        holder = [None]
        mod.set_axon_ntff_profile_hook = lambda h: holder.__setitem__(0, h)
        mod.get_axon_ntff_profile_hook = lambda: holder[0]
        sys.modules["antenv.axon_hooks"] = mod
        from trn_agent_boot.trn_boot import _ntff_profile_via_ctypes

        so = "/opt/axon/libaxon_pjrt.so"
        if os.path.exists(so):
            holder[0] = _ntff_profile_via_ctypes(so)
    except Exception:
        pass


def _build_module():
    nc = bacc.Bacc()
    Alu = mybir.AluOpType

    xT = nc.dram_tensor("xT", [1, C * XLEN], F32, kind="ExternalInput")
    S1 = nc.dram_tensor("S1", [128, P], F32, kind="ExternalInput")
    S2 = nc.dram_tensor("S2", [128, P], F32, kind="ExternalInput")
    UB = nc.dram_tensor("UB", [128, 2], F32, kind="ExternalInput")
    BC = nc.dram_tensor("BC", [128, COUT], F32, kind="ExternalInput")
    PM = nc.dram_tensor("PM", [1, 128], F32, kind="ExternalInput")
    M1 = nc.dram_tensor("M1", [128, COUT], F16, kind="ExternalInput")
    M2 = nc.dram_tensor("M2", [128, COUT], F16, kind="ExternalInput")
    Y = nc.dram_tensor("Y", [HO * WO, COUT], F32, kind="ExternalOutput")

    with tile.TileContext(nc) as tc, ExitStack() as ctx:
        const = ctx.enter_context(tc.tile_pool(name="const", bufs=1))
        xbp = ctx.enter_context(tc.tile_pool(name="xbp", bufs=2, space="PSUM"))
        xbs = ctx.enter_context(tc.tile_pool(name="xbs", bufs=3))
        accp = ctx.enter_context(tc.tile_pool(name="accp", bufs=1))
        prodp = ctx.enter_context(tc.tile_pool(name="prodp", bufs=4))
        tps = ctx.enter_context(tc.tile_pool(name="tps", bufs=2, space="PSUM"))
        tsb = ctx.enter_context(tc.tile_pool(name="tsb", bufs=2))

        xT_sb = const.tile([1, C * XLEN], F32)
        nc.gpsimd.dma_start(out=xT_sb[:, :], in_=xT[:, :])
        S1_sb = const.tile([128, P], F32)
        nc.gpsimd.dma_start(out=S1_sb[:, :], in_=S1[:, :])
        S2_sb = const.tile([128, P], F32)
        nc.gpsimd.dma_start(out=S2_sb[:, :], in_=S2[:, :])
        UB_sb = const.tile([128, 2], F32)
        nc.gpsimd.dma_start(out=UB_sb[:, :], in_=UB[:, :])
        BC_sb = const.tile([128, COUT], F32)
        nc.gpsimd.dma_start(out=BC_sb[:, :], in_=BC[:, :])
        PM_sb = const.tile([1, 128], F32)
        nc.gpsimd.dma_start(out=PM_sb[:, :], in_=PM[:, :])
        M1_sb = const.tile([128, COUT], F16)
        nc.gpsimd.dma_start(out=M1_sb[:, :], in_=M1[:, :])
        M2_sb = const.tile([128, COUT], F16)
        nc.gpsimd.dma_start(out=M2_sb[:, :], in_=M2[:, :])

        # accW = two independent copies of [K1 | K2] accs side by side, fp16;
        # partitions = [A(+x)|B(-x)].  One TT folds TWO iterations' products.
        accW = accp.tile([128, 4 * FD], F16)
        nc.gpsimd.memset(accW[:, :], 0.0)
        for h in range(4):
            nc.vector.tensor_scalar(
                out=accW[:, h * FD : (h + 1) * FD],
                in0=accW[:, h * FD : (h + 1) * FD],
                scalar1=UB_sb[:, h % 2 : h % 2 + 1], scalar2=None, op0=Alu.add,
            )
        pending = []  # software pipeline: fold product pairs one TT late
        pp = None

        for ci in range(C):
            # broadcast row ci of xT to [ +x (64 parts) ; -x (64 parts) ]
            xq = xbp.tile([128, XLEN], F32)
            for s, e in ((0, 512), (512, 1024), (1024, XLEN)):
                nc.tensor.matmul(
                    xq[:, s:e], lhsT=PM_sb[:, :], rhs=xT_sb[0:1, ci * XLEN + s : ci * XLEN + e],
                    start=True, stop=True,
                )
            # fp16 staging, two parities so every tap window is 4B-aligned
            xbE = xbs.tile([128, XLEN], F16, tag="xbE")
            nc.scalar.copy(out=xbE[:, :], in_=xq[:, :])
            xbO = xbs.tile([128, XLEN - 1], F16, tag="xbO")
            nc.scalar.copy(out=xbO[:, :], in_=xq[:, 1:XLEN])

            for t in range(9):
                i, j = divmod(t, 3)
                off = i * W + j
                p = t * C + ci
                # 30x30 output window at tap offset, row stride W (even base)
                if off % 2 == 0:
                    src = xbE[:, off : off + HO * W]
                else:
                    src = xbO[:, off - 1 : off - 1 + HO * W]
                in0 = src.rearrange("q (a b) -> q a b", b=W)[:, :, :WO]
                k = ci * 9 + t
                if k % 2 == 0:
                    pp = prodp.tile([128, 4 * FD], F16)
                base = (k % 2) * 2 * FD
                for lo, S_sb in ((0, S1_sb), (FD, S2_sb)):
                    nc.vector.tensor_scalar(
                        out=pp[:, base + lo : base + lo + FD].rearrange(
                            "q (a b) -> q a b", a=HO),
                        in0=in0, scalar1=S_sb[:, p : p + 1],
                        scalar2=None, op0=Alu.mult,
                    )
                if k % 2 == 1:
                    pending.append(pp)
                if len(pending) > 1:
                    q = pending.pop(0)
                    nc.vector.tensor_tensor(
                        accW[:, :], q[:, :], accW[:, :], Alu.max,
                    )

        for q in pending:
            nc.vector.tensor_tensor(
                accW[:, :], q[:, :], accW[:, :], Alu.max,
            )
        acc12 = accW[:, 0 : 2 * FD]
        nc.vector.tensor_tensor(
            acc12, accW[:, 2 * FD : 4 * FD], acc12, Alu.max,
        )

        # Combine + transpose in one PE op per 128-pos chunk:
        #   pt = acc1_chunk.T @ [I;-I]  +  acc2_chunk.T @ [-I;I]
        #      = (accA1-accB1) - (accA2-accB2), position-major [cw, 64].
        # Then add the partition-replicated bias and DMA the chunk out.
        for c0 in range(0, FD, 128):
            cw = min(128, FD - c0)
            pt = tps.tile([128, COUT], F32)
            nc.tensor.matmul(pt[:cw, :], lhsT=accW[:, c0 : c0 + cw], rhs=M1_sb[:, :],
                             start=True, stop=False)
            nc.tensor.matmul(pt[:cw, :], lhsT=accW[:, FD + c0 : FD + c0 + cw], rhs=M2_sb[:, :],
                             start=False, stop=True)
            ysb = tsb.tile([128, COUT], F32)
            nc.vector.tensor_tensor(ysb[:cw, :], pt[:cw, :], BC_sb[:cw, :], Alu.add)
            nc.sync.dma_start(out=Y[c0 : c0 + cw, :], in_=ysb[:cw, :])
    nc.finalize()
    return nc


def _host_prep(x, k1, k2, bias):
    x = np.ascontiguousarray(np.asarray(x, dtype=np.float32))
    K1 = np.exp(np.asarray(k1, np.float32).reshape(P, COUT))
    K2 = np.exp(np.asarray(k2, np.float32).reshape(P, COUT))
    S1 = np.vstack([K1.T, K1.T]).astype(np.float32)          # [128, 288]
    S2 = np.vstack([K2.T, K2.T]).astype(np.float32)
    U1 = 0.1 * K1.max(axis=0)
    U2 = 0.1 * K2.max(axis=0)
    UB = np.stack([np.concatenate([U1, U1]), np.concatenate([U2, U2])], axis=1)
    UB = np.ascontiguousarray(UB, np.float32)                # [128, 2]
    BC = np.tile(np.asarray(bias, np.float32).reshape(1, COUT), (128, 1))
    PM = np.concatenate([np.ones(64, np.float32), -np.ones(64, np.float32)]).reshape(1, 128)
    M1 = np.vstack([np.eye(COUT, dtype=np.float16), -np.eye(COUT, dtype=np.float16)])
    M2 = np.ascontiguousarray(-M1)
    shared = dict(S1=S1, S2=S2, UB=UB, BC=np.ascontiguousarray(BC),
                  PM=np.ascontiguousarray(PM), M1=np.ascontiguousarray(M1), M2=M2)
    in_maps = []
    for n in range(N_CORES):
        xT = np.zeros((C, XLEN), np.float32)
        xT[:, :NPIX] = x[n].reshape(NPIX, C).T
        in_maps.append({"xT": xT.reshape(1, C * XLEN), **shared})
    return in_maps


def kernel(x, k1, k2, bias):
    global last_results
    if "nc" not in _cache:
        _cache["nc"] = _build_module()
    nc = _cache["nc"]
    in_maps = _host_prep(x, k1, k2, bias)
    trace = bool(int(os.environ.get("KTRACE", "0")))
    if trace:
        _ensure_axon_ntff_hook()
    res = run_bass_kernel_spmd(
        nc, in_maps, core_ids=list(range(N_CORES)), trace=trace,
    )
    last_results = res
    y = np.stack([r["Y"].reshape(HO, WO, COUT) for r in res.results], axis=0)
    return y.astype(np.float32)



# revision 18
# speedup vs baseline: 6.6222x; 6.6222x over previous
"""Bipolar morphological conv2d kernel for Trainium2 (8 NeuronCores).

Math: reference computes, per output position and out-channel c,
    y = m(lp1,K1) - m(lp1,K2) - m(lp2,K1) + m(lp2,K2) + bias
with m(logp, k)[c] = exp(max_p(logp_p + k_pc)), lp1 = log(max(x, .1)),
lp2 = log(max(-x, .1)).

Device algorithm (data-parallel, one batch image per core): the 288-tap
max-plus reduction is evaluated as a tight LSE (p-norm, t=112) over each
3x1 column group of the 3x3 window (96 entries: 3 rows x 32 channels),
turning the heavy reduction into THREE K=96 TensorE matmuls, followed by
an EXACT max over the 3 column groups in log domain (DVE fp16).  A
per-rhs-column normalizer M3q (itself a sigma=32 LSE, computed with a
ones-matmul) keeps every exponential in fp32 range; its value cancels
exactly in the algebra, so only over/underflow matters, not its accuracy.

Layout: channel-major pixel rows.  xT3 [96, 1024] holds the 3 row-shifted
copies of xT [32, 1024] (partition 32g+ci = channel ci shifted g rows),
so a column-group matmul contracts all 96 entries in one instruction and
tap shifts become free-dim column offsets (multiples of 1).  Out-channels
of K1|K2 are stacked on the 128 output partitions, so one matmul chain
serves both kernels.  Final exp folds the per-channel max-k and the
30x30 window selection; a pair of +/-I matmuls transposes to
position-major and combines the 4 morphs with their signs.
"""

import os
from contextlib import ExitStack

import numpy as np

import concourse.bass as bass
import concourse.mybir as mybir
from concourse import bacc
import concourse.tile as tile
from concourse.bass_utils import run_bass_kernel_spmd

N_CORES = 8
H = W = C = 32
COUT = 64
HO = WO = 30
NPIX = H * W            # 1024
XW = 1024               # working row width (pixels)
XIN = 1088              # input row width (1024 + 64 pad for row shifts)
ACW = 960               # accumulator width (30 rows x 32 cols)
POSW = 958              # last used pos col is 29*32+29 = 957

SIG1 = 8.0              # stage-1 normalizer LSE sharpness (fits ACT Ln window)
SIG2 = 32.0             # stage-2 normalizer refinement sharpness
T = 112.0               # main LSE sharpness
G = 6.0                 # fixed global scale bound (|x| < 6 for N(0,1) data)
LG = float(np.log(G))
CSH = 216.0             # fp16 recentering shift for the log-domain combine
GCAP = 38.5             # cap on per-channel ln-rescale (ACT Ln window ~|44|)

F32 = mybir.dt.float32
F16 = mybir.dt.float16
_cache: dict = {}
last_results = None


def _ensure_axon_ntff_hook():
    """The trimmed agent image lacks antenv.axon_hooks; recreate it so
    run_bass_kernel_spmd(trace=True) can capture NTFF profiles. No-op on
    failure (tracing then just degrades)."""
    import sys
    import types

    try:
        import antenv.axon_hooks  # noqa: F401
        return
    except ImportError:
        pass
    try:
        mod = types.ModuleType("antenv.axon_hooks")
        holder = [None]
        mod.set_axon_ntff_profile_hook = lambda h: holder.__setitem__(0, h)
        mod.get_axon_ntff_profile_hook = lambda: holder[0]
        sys.modules["antenv.axon_hooks"] = mod
        from trn_agent_boot.trn_boot import _ntff_profile_via_ctypes

        so = "/opt/axon/libaxon_pjrt.so"
        if os.path.exists(so):
            holder[0] = _ntff_profile_via_ctypes(so)
    except Exception:
        pass


def _build_module():
    nc = bacc.Bacc()
    Alu = mybir.AluOpType
    Act = mybir.ActivationFunctionType

    xT = nc.dram_tensor("xT", [C, XIN], F32, kind="ExternalInput")
    K3 = nc.dram_tensor("K3", [96, 384], F32, kind="ExternalInput")
    MM = nc.dram_tensor("MM", [128, 128], F16, kind="ExternalInput")
    EXM = nc.dram_tensor("EXM", [128, 1], F32, kind="ExternalInput")
    SC = nc.dram_tensor("SC", [128, 1], F32, kind="ExternalInput")
    AFL = nc.dram_tensor("AFL", [128, 1], F32, kind="ExternalInput")
    ONES = nc.dram_tensor("ONES", [96, 128], F32, kind="ExternalInput")
    BC = nc.dram_tensor("BC", [128, COUT], F32, kind="ExternalInput")
    Y = nc.dram_tensor("Y", [HO * WO, COUT], F32, kind="ExternalOutput")

    with tile.TileContext(nc) as tc, ExitStack() as ctx:
        const = ctx.enter_context(tc.tile_pool(name="const", bufs=1))
        sp = ctx.enter_context(tc.tile_pool(name="sp", bufs=2))
        losb = ctx.enter_context(tc.tile_pool(name="losb", bufs=3))
        accp = ctx.enter_context(tc.tile_pool(name="accp", bufs=2))
        vsb = ctx.enter_context(tc.tile_pool(name="vsb", bufs=2))
        msb = ctx.enter_context(tc.tile_pool(name="msb", bufs=2))
        ysb = ctx.enter_context(tc.tile_pool(name="ysb", bufs=2))
        psS = ctx.enter_context(tc.tile_pool(name="psS", bufs=1, space="PSUM"))
        psO = ctx.enter_context(tc.tile_pool(name="psO", bufs=2, space="PSUM"))
        psPT = ctx.enter_context(tc.tile_pool(name="psPT", bufs=2, space="PSUM"))

        # ---- constants / input staging ----
        xT3 = const.tile([96, XW], F32)
        for g in range(3):
            nc.gpsimd.dma_start(out=xT3[32 * g : 32 * g + 32, :],
                                in_=xT[:, 32 * g : 32 * g + XW])
        K3_sb = const.tile([96, 384], F32)
        nc.gpsimd.dma_start(out=K3_sb[:, :], in_=K3[:, :])
        MM_sb = const.tile([128, 128], F16)
        nc.gpsimd.dma_start(out=MM_sb[:, :], in_=MM[:, :])
        EXM_sb = const.tile([128, 1], F32)
        nc.gpsimd.dma_start(out=EXM_sb[:, :], in_=EXM[:, :])
        SC_sb = const.tile([128, 1], F32)
        nc.gpsimd.dma_start(out=SC_sb[:, :], in_=SC[:, :])
        AFL_sb = const.tile([128, 1], F32)
        nc.gpsimd.dma_start(out=AFL_sb[:, :], in_=AFL[:, :])
        ONES_sb = const.tile([96, 128], F32)
        nc.gpsimd.dma_start(out=ONES_sb[:, :], in_=ONES[:, :])
        BC_sb = const.tile([128, COUT], F32)
        nc.gpsimd.dma_start(out=BC_sb[:, :], in_=BC[:, :])
        B8_sb = const.tile([96, 1], F32)
        nc.gpsimd.memset(B8_sb[:, :], -SIG1 * LG)
        B32_sb = const.tile([96, 1], F32)
        nc.gpsimd.memset(B32_sb[:, :], -SIG2 * LG)
        B96_sb = const.tile([96, 1], F32)
        nc.gpsimd.memset(B96_sb[:, :], -T * LG)

        m_tiles = []
        for sgn in (1.0, -1.0):
            # X = max(sgn*x, 0.1); lp = ln X
            X3 = sp.tile([96, XW], F32, tag="X3")
            nc.vector.tensor_scalar(out=X3[:, :], in0=xT3[:, :],
                                    scalar1=sgn, scalar2=0.1,
                                    op0=Alu.mult, op1=Alu.max)
            lp3 = sp.tile([96, XW], F32, tag="lp3")
            nc.scalar.activation(lp3[:, :], X3[:, :], Act.Ln)
            # stage-1 normalizer: E8 = (X/G)^8, M8 = lG + L8/8
            E8 = sp.tile([96, XW], F32, tag="E8")
            nc.scalar.activation(E8[:, :], lp3[:, :], Act.Exp,
                                 bias=B8_sb[:, 0:1], scale=SIG1)
            S8p = psS.tile([128, XW], F32, tag="Sp")
            for c0 in (0, 512):
                nc.tensor.matmul(S8p[:, c0 : c0 + 512], lhsT=ONES_sb[:, :],
                                 rhs=E8[:, c0 : c0 + 512], start=True, stop=True)
            L8 = sp.tile([128, XW], F32, tag="L8")
            nc.scalar.activation(L8[:, :], S8p[:, :], Act.Ln)
            # stage-2 refinement: E32 = exp(SIG2*(lp - M8)), in-window by design
            d8 = sp.tile([96, XW], F32, tag="d8")
            nc.vector.scalar_tensor_tensor(out=d8[:, :], in0=L8[0:96, :],
                                           scalar=-1.0 / SIG1, in1=lp3[:, :],
                                           op0=Alu.mult, op1=Alu.add)
            E32 = sp.tile([96, XW], F32, tag="E32")
            nc.scalar.activation(E32[:, :], d8[:, :], Act.Exp,
                                 bias=B32_sb[:, 0:1], scale=SIG2)
            S32p = psS.tile([128, XW], F32, tag="Sp")
            for c0 in (0, 512):
                nc.tensor.matmul(S32p[:, c0 : c0 + 512], lhsT=ONES_sb[:, :],
                                 rhs=E32[:, c0 : c0 + 512], start=True, stop=True)
            L32 = sp.tile([128, XW], F32, tag="L32")
            nc.scalar.activation(L32[:, :], S32p[:, :], Act.Ln)
            # combined normalizer LS0 = 4*L8 + L32  (M3q = lG + LS0/32)
            LS0 = sp.tile([128, XW], F32, tag="LS0")
            nc.vector.scalar_tensor_tensor(out=LS0[:, :], in0=L8[:, :],
                                           scalar=SIG2 / SIG1, in1=L32[:, :],
                                           op0=Alu.mult, op1=Alu.add)
            # T16 parity copies: (T/SIG2)*LS0 + CSH in fp16
            T16e = sp.tile([128, XW], F16, tag="T16e")
            nc.vector.tensor_scalar(out=T16e[:, :], in0=LS0[:, :],
                                    scalar1=T / SIG2, scalar2=CSH,
                                    op0=Alu.mult, op1=Alu.add)
            T16o = sp.tile([128, XW - 2], F16, tag="T16o")
            nc.vector.tensor_scalar(out=T16o[:, :], in0=LS0[:, 1 : XW - 1],
                                    scalar1=T / SIG2, scalar2=CSH,
                                    op0=Alu.mult, op1=Alu.add)
            # d96 = lp - LS0/SIG2 ; E96 = exp(T*(d96 - lG)) = exp(T*(lp - M3q))
            d96 = sp.tile([96, XW], F32, tag="d96")
            nc.vector.scalar_tensor_tensor(out=d96[:, :], in0=LS0[0:96, :],
                                           scalar=-1.0 / SIG2, in1=lp3[:, :],
                                           op0=Alu.mult, op1=Alu.add)
            E96 = sp.tile([96, XW], F32, tag="E96")
            nc.scalar.activation(E96[:, :], d96[:, :], Act.Exp,
                                 bias=B96_sb[:, 0:1], scale=T)

            # per column group j: K=96 matmul, shifted ln, fp16 combine
            acc = accp.tile([128, ACW], F16, tag="acc")
            for j in range(3):
                Oj = psO.tile([128, XW], F32, tag="Oj")
                nc.tensor.matmul(Oj[:, 0:512], lhsT=K3_sb[:, 128 * j : 128 * j + 128],
                                 rhs=E96[:, j : j + 512], start=True, stop=True)
                nc.tensor.matmul(Oj[:, 512:ACW], lhsT=K3_sb[:, 128 * j : 128 * j + 128],
                                 rhs=E96[:, j + 512 : j + ACW], start=True, stop=True)
                LoS = losb.tile([128, ACW], F16, tag="LoS")
                nc.scalar.activation(LoS[:, :], Oj[:, 0:ACW], Act.Ln,
                                     scale=SC_sb[:, 0:1])
                t16 = T16e if j % 2 == 0 else T16o
                toff = j if j % 2 == 0 else j - 1
                if j == 0:
                    nc.vector.tensor_tensor(
                        acc[:, 0:POSW], LoS[:, 0:POSW],
                        t16[:, toff : toff + POSW], Alu.add)
                else:
                    V = vsb.tile([128, POSW], F16, tag="V")
                    nc.vector.tensor_tensor(
                        V[:, :], LoS[:, 0:POSW],
                        t16[:, toff : toff + POSW], Alu.add)
                    nc.vector.tensor_tensor(
                        acc[:, 0:POSW], V[:, :], acc[:, 0:POSW], Alu.max)
            # exact morph floor U_c = 0.1*exp(Mk_c)  (kills ln-saturation tails)
            nc.vector.tensor_scalar(out=acc[:, 0:POSW], in0=acc[:, 0:POSW],
                                    scalar1=AFL_sb[:, 0:1], scalar2=None,
                                    op0=Alu.max)
            # m = exp(acc/T + Mk + lG - (CSH+g_c)/T), windowed -> compact 900
            m = msb.tile([128, HO * WO], F16, tag="m")
            nc.scalar.activation(
                m.rearrange("q (a b) -> q a b", a=HO),
                acc.rearrange("q (a b) -> q a b", b=W)[:, :, :WO],
                Act.Exp, bias=EXM_sb[:, 0:1], scale=1.0 / T)
            m_tiles.append(m)

        # combine + transpose: pt[pos, c] = m1.T @ [I;-I] + m2.T @ [-I;I]
        m1, m2 = m_tiles
        for c0 in range(0, HO * WO, 128):
            cw = min(128, HO * WO - c0)
            pt = psPT.tile([128, COUT], F32, tag="pt")
            nc.tensor.matmul(pt[:cw, :], lhsT=m1[:, c0 : c0 + cw],
                             rhs=MM_sb[:, 0:COUT], start=True, stop=False)
            nc.tensor.matmul(pt[:cw, :], lhsT=m2[:, c0 : c0 + cw],
                             rhs=MM_sb[:, COUT:128], start=False, stop=True)
            yt = ysb.tile([128, COUT], F32, tag="yt")
            nc.vector.tensor_tensor(yt[:cw, :], pt[:cw, :], BC_sb[:cw, :], Alu.add)
            nc.sync.dma_start(out=Y[c0 : c0 + cw, :], in_=yt[:cw, :])
    nc.finalize()
    return nc


def _host_prep(x, k1, k2, bias):
    x = np.ascontiguousarray(np.asarray(x, dtype=np.float32))
    k1 = np.asarray(k1, np.float32).reshape(3, 3, C, COUT)
    k2 = np.asarray(k2, np.float32).reshape(3, 3, C, COUT)
    Mk1 = k1.reshape(-1, COUT).max(axis=0)
    Mk2 = k2.reshape(-1, COUT).max(axis=0)
    K3 = np.zeros((96, 384), np.float32)
    for j in range(3):
        for g in range(3):
            K3[32 * g : 32 * g + 32, 128 * j : 128 * j + 64] = \
                np.exp(T * (k1[g, j] - Mk1))
            K3[32 * g : 32 * g + 32, 128 * j + 64 : 128 * j + 128] = \
                np.exp(T * (k2[g, j] - Mk2))
    I64 = np.eye(COUT, dtype=np.float16)
    MM = np.zeros((128, 128), np.float16)
    MM[0:64, 0:COUT] = I64
    MM[64:128, 0:COUT] = -I64
    MM[0:64, COUT:128] = -I64
    MM[64:128, COUT:128] = I64
    rng1 = Mk1 - k1.reshape(-1, COUT).min(axis=0)
    rng2 = Mk2 - k2.reshape(-1, COUT).min(axis=0)
    gc = np.minimum((16.2 + T * np.concatenate([rng1, rng2]) - 5.0) / 2.0, GCAP)
    Mk = np.concatenate([Mk1, Mk2])
    EXM = (Mk + LG - (CSH + gc) / T).reshape(128, 1)
    SC = np.exp(gc).reshape(128, 1)
    AFL = (T * (np.log(0.1) - LG) + CSH + gc).reshape(128, 1)
    ONES = np.ones((96, 128), np.float32)
    BC = np.tile(np.asarray(bias, np.float32).reshape(1, COUT), (128, 1))
    shared = dict(K3=K3, MM=MM, EXM=EXM.astype(np.float32),
                  SC=SC.astype(np.float32), AFL=AFL.astype(np.float32),
                  ONES=ONES, BC=np.ascontiguousarray(BC))
    in_maps = []
    for n in range(N_CORES):
        xT = np.zeros((C, XIN), np.float32)
        xT[:, :NPIX] = x[n].reshape(NPIX, C).T
        in_maps.append({"xT": xT, **shared})
    return in_maps


def kernel(x, k1, k2, bias):
    global last_results
    if "nc" not in _cache:
        _cache["nc"] = _build_module()
    nc = _cache["nc"]
    in_maps = _host_prep(x, k1, k2, bias)
    trace = bool(int(os.environ.get("KTRACE", "0")))
    if trace:
        _ensure_axon_ntff_hook()
    res = run_bass_kernel_spmd(
        nc, in_maps, core_ids=list(range(N_CORES)), trace=trace,
    )
    last_results = res
    y = np.stack([r["Y"].reshape(HO, WO, COUT) for r in res.results], axis=0)
    return y.astype(np.float32)


# revision 21
# speedup vs baseline: 7.4391x; 1.1234x over previous
"""Bipolar morphological conv2d kernel for Trainium2 (8 NeuronCores).

Math: reference computes, per output position and out-channel c,
    y = m(lp1,K1) - m(lp1,K2) - m(lp2,K1) + m(lp2,K2) + bias
with m(logp, k)[c] = exp(max_p(logp_p + k_pc)), lp1 = log(max(x, .1)),
lp2 = log(max(-x, .1)).

Device algorithm (data-parallel, one batch image per core): the 288-tap
max-plus reduction is evaluated as a tight LSE (p-norm, t=112) over each
3x1 column group of the 3x3 window (96 entries: 3 rows x 32 channels),
turning the heavy reduction into THREE K=96 TensorE matmuls, followed by
an EXACT max over the 3 column groups in log domain (DVE fp16).  A
per-rhs-column normalizer M3q (itself a sigma=32 LSE, computed with a
ones-matmul) keeps every exponential in fp32 range; its value cancels
exactly in the algebra, so only over/underflow matters, not its accuracy.

Layout: channel-major pixel rows.  xT3 [96, 1024] holds the 3 row-shifted
copies of xT [32, 1024] (partition 32g+ci = channel ci shifted g rows),
so a column-group matmul contracts all 96 entries in one instruction and
tap shifts become free-dim column offsets (multiples of 1).  Out-channels
of K1|K2 are stacked on the 128 output partitions, so one matmul chain
serves both kernels.  Final exp folds the per-channel max-k and the
30x30 window selection; a pair of +/-I matmuls transposes to
position-major and combines the 4 morphs with their signs.
"""

import os
from contextlib import ExitStack

import numpy as np

import concourse.bass as bass
import concourse.mybir as mybir
from concourse import bacc
import concourse.tile as tile
from concourse.bass_utils import run_bass_kernel_spmd

N_CORES = 8
H = W = C = 32
COUT = 64
HO = WO = 30
NPIX = H * W            # 1024
XW = 1024               # working row width (pixels)
XIN = 1088              # input row width (1024 + 64 pad for row shifts)
ACW = 960               # accumulator width (30 rows x 32 cols)
POSW = 958              # last used pos col is 29*32+29 = 957

SIG1 = 8.0              # stage-1 normalizer LSE sharpness (fits ACT Ln window)
SIG2 = 32.0             # stage-2 normalizer refinement sharpness
T = 112.0               # main LSE sharpness
G = 6.0                 # fixed global scale bound (|x| < 6 for N(0,1) data)
LG = float(np.log(G))
CSH = 216.0             # fp16 recentering shift for the log-domain combine
GCAP = 38.5             # cap on per-channel ln-rescale (ACT Ln window ~|44|)

F32 = mybir.dt.float32
F16 = mybir.dt.float16
_cache: dict = {}
last_results = None


def _ensure_axon_ntff_hook():
    """The trimmed agent image lacks antenv.axon_hooks; recreate it so
    run_bass_kernel_spmd(trace=True) can capture NTFF profiles. No-op on
    failure (tracing then just degrades)."""
    import sys
    import types

    try:
        import antenv.axon_hooks  # noqa: F401
        return
    except ImportError:
        pass
    try:
        mod = types.ModuleType("antenv.axon_hooks")
        holder = [None]
        mod.set_axon_ntff_profile_hook = lambda h: holder.__setitem__(0, h)
        mod.get_axon_ntff_profile_hook = lambda: holder[0]
        sys.modules["antenv.axon_hooks"] = mod
        from trn_agent_boot.trn_boot import _ntff_profile_via_ctypes

        so = "/opt/axon/libaxon_pjrt.so"
        if os.path.exists(so):
            holder[0] = _ntff_profile_via_ctypes(so)
    except Exception:
        pass


def _build_module():
    nc = bacc.Bacc()
    Alu = mybir.AluOpType
    Act = mybir.ActivationFunctionType

    xT = nc.dram_tensor("xT", [C, XIN], F32, kind="ExternalInput")
    K3 = nc.dram_tensor("K3", [96, 384], F32, kind="ExternalInput")
    MM = nc.dram_tensor("MM", [128, 128], F16, kind="ExternalInput")
    EXM = nc.dram_tensor("EXM", [128, 1], F32, kind="ExternalInput")
    SC = nc.dram_tensor("SC", [128, 1], F32, kind="ExternalInput")
    AFL = nc.dram_tensor("AFL", [128, 1], F32, kind="ExternalInput")
    ONES = nc.dram_tensor("ONES", [96, 128], F32, kind="ExternalInput")
    BC = nc.dram_tensor("BC", [128, COUT], F32, kind="ExternalInput")
    Y = nc.dram_tensor("Y", [HO * WO, COUT], F32, kind="ExternalOutput")

    with tile.TileContext(nc) as tc, ExitStack() as ctx:
        const = ctx.enter_context(tc.tile_pool(name="const", bufs=1))
        sp = ctx.enter_context(tc.tile_pool(name="sp", bufs=1))
        losb = ctx.enter_context(tc.tile_pool(name="losb", bufs=3))
        accp = ctx.enter_context(tc.tile_pool(name="accp", bufs=1))
        vsb = ctx.enter_context(tc.tile_pool(name="vsb", bufs=1))
        msb = ctx.enter_context(tc.tile_pool(name="msb", bufs=1))
        ysb = ctx.enter_context(tc.tile_pool(name="ysb", bufs=2))
        psS = ctx.enter_context(tc.tile_pool(name="psS", bufs=2, space="PSUM"))
        psO = ctx.enter_context(tc.tile_pool(name="psO", bufs=2, space="PSUM"))

        # ---- constants / input staging ----
        xT3 = const.tile([96, XW], F32)
        for g in range(3):
            nc.gpsimd.dma_start(out=xT3[32 * g : 32 * g + 32, :],
                                in_=xT[:, 32 * g : 32 * g + XW])
        K3_sb = const.tile([96, 384], F32)
        nc.gpsimd.dma_start(out=K3_sb[:, :], in_=K3[:, :])
        MM_sb = const.tile([128, 128], F16)
        nc.gpsimd.dma_start(out=MM_sb[:, :], in_=MM[:, :])
        EXM_sb = const.tile([128, 1], F32)
        nc.gpsimd.dma_start(out=EXM_sb[:, :], in_=EXM[:, :])
        SC_sb = const.tile([128, 1], F32)
        nc.gpsimd.dma_start(out=SC_sb[:, :], in_=SC[:, :])
        AFL_sb = const.tile([128, 1], F32)
        nc.gpsimd.dma_start(out=AFL_sb[:, :], in_=AFL[:, :])
        ONES_sb = const.tile([96, 128], F32)
        nc.gpsimd.dma_start(out=ONES_sb[:, :], in_=ONES[:, :])
        BC_sb = const.tile([128, COUT], F32)
        nc.gpsimd.dma_start(out=BC_sb[:, :], in_=BC[:, :])
        B8_sb = const.tile([96, 1], F32)
        nc.gpsimd.memset(B8_sb[:, :], -SIG1 * LG)
        B32_sb = const.tile([96, 1], F32)
        nc.gpsimd.memset(B32_sb[:, :], -SIG2 * LG)
        B96_sb = const.tile([96, 1], F32)
        nc.gpsimd.memset(B96_sb[:, :], -T * LG)

        m_tiles = []
        S = [dict(), dict()]
        for i, sgn in ((0, 1.0), (1, -1.0)):
            X3 = sp.tile([96, XW], F32, tag=f"X3{i}")
            nc.vector.tensor_scalar(out=X3[:, :], in0=xT3[:, :],
                                    scalar1=sgn, scalar2=0.1,
                                    op0=Alu.mult, op1=Alu.max)
            S[i]["X3"] = X3
        for i in (0, 1):
            lp3 = sp.tile([96, XW], F32, tag=f"lp3{i}")
            nc.scalar.activation(lp3[:, :], S[i]["X3"][:, :], Act.Ln)
            S[i]["lp3"] = lp3
        for i in (0, 1):
            E8 = sp.tile([96, XW], F32, tag=f"E8{i}")
            nc.scalar.activation(E8[:, :], S[i]["lp3"][:, :], Act.Exp,
                                 bias=B8_sb[:, 0:1], scale=SIG1)
            S[i]["E8"] = E8
        for i in (0, 1):
            S8p = psS.tile([128, XW], F32, tag="Sp")
            for c0 in (0, 512):
                nc.tensor.matmul(S8p[:, c0 : c0 + 512], lhsT=ONES_sb[:, :],
                                 rhs=S[i]["E8"][:, c0 : c0 + 512],
                                 start=True, stop=True)
            S[i]["S8p"] = S8p
        for i in (0, 1):
            L8 = sp.tile([128, XW], F32, tag=f"L8{i}")
            nc.scalar.activation(L8[:, :], S[i]["S8p"][:, :], Act.Ln)
            S[i]["L8"] = L8
        for i in (0, 1):
            d8 = sp.tile([96, XW], F32, tag=f"d8{i}")
            nc.vector.scalar_tensor_tensor(out=d8[:, :], in0=S[i]["L8"][0:96, :],
                                           scalar=-1.0 / SIG1, in1=S[i]["lp3"][:, :],
                                           op0=Alu.mult, op1=Alu.add)
            S[i]["d8"] = d8
        for i in (0, 1):
            E32 = sp.tile([96, XW], F32, tag=f"E32{i}")
            nc.scalar.activation(E32[:, :], S[i]["d8"][:, :], Act.Exp,
                                 bias=B32_sb[:, 0:1], scale=SIG2)
            S[i]["E32"] = E32
        for i in (0, 1):
            S32p = psS.tile([128, XW], F32, tag="Sp")
            for c0 in (0, 512):
                nc.tensor.matmul(S32p[:, c0 : c0 + 512], lhsT=ONES_sb[:, :],
                                 rhs=S[i]["E32"][:, c0 : c0 + 512],
                                 start=True, stop=True)
            S[i]["S32p"] = S32p
        for i in (0, 1):
            L32 = sp.tile([128, XW], F32, tag=f"L32{i}")
            nc.scalar.activation(L32[:, :], S[i]["S32p"][:, :], Act.Ln)
            S[i]["L32"] = L32
        for i in (0, 1):
            LS0 = sp.tile([128, XW], F32, tag=f"LS0{i}")
            nc.vector.scalar_tensor_tensor(out=LS0[:, :], in0=S[i]["L8"][:, :],
                                           scalar=SIG2 / SIG1, in1=S[i]["L32"][:, :],
                                           op0=Alu.mult, op1=Alu.add)
            S[i]["LS0"] = LS0
        for i in (0, 1):
            d96 = sp.tile([96, XW], F32, tag=f"d96{i}")
            nc.vector.scalar_tensor_tensor(out=d96[:, :], in0=S[i]["LS0"][0:96, :],
                                           scalar=-1.0 / SIG2, in1=S[i]["lp3"][:, :],
                                           op0=Alu.mult, op1=Alu.add)
            S[i]["d96"] = d96
        for i in (0, 1):
            E96 = sp.tile([96, XW], F32, tag=f"E96{i}")
            nc.scalar.activation(E96[:, :], S[i]["d96"][:, :], Act.Exp,
                                 bias=B96_sb[:, 0:1], scale=T)
            S[i]["E96"] = E96
        for i in (0, 1):
            T16e = sp.tile([128, XW], F16, tag=f"T16e{i}")
            nc.vector.tensor_scalar(out=T16e[:, :], in0=S[i]["LS0"][:, :],
                                    scalar1=T / SIG2, scalar2=CSH,
                                    op0=Alu.mult, op1=Alu.add)
            T16o = sp.tile([128, XW - 2], F16, tag=f"T16o{i}")
            nc.vector.tensor_scalar(out=T16o[:, :], in0=S[i]["LS0"][:, 1 : XW - 1],
                                    scalar1=T / SIG2, scalar2=CSH,
                                    op0=Alu.mult, op1=Alu.add)
            S[i]["T16e"], S[i]["T16o"] = T16e, T16o
            acc = accp.tile([128, ACW], F16, tag=f"acc{i}")
            S[i]["acc"] = acc
        for j in range(3):
            for i in (0, 1):
                Oj = psO.tile([128, XW], F32, tag="Oj")
                E96 = S[i]["E96"]
                nc.tensor.matmul(Oj[:, 0:512], lhsT=K3_sb[:, 128 * j : 128 * j + 128],
                                 rhs=E96[:, j : j + 512], start=True, stop=True)
                nc.tensor.matmul(Oj[:, 512:ACW], lhsT=K3_sb[:, 128 * j : 128 * j + 128],
                                 rhs=E96[:, j + 512 : j + ACW], start=True, stop=True)
                S[i]["Oj"] = Oj
            for i in (0, 1):
                LoS = losb.tile([128, ACW], F16, tag=f"LoS{i}")
                nc.scalar.activation(LoS[:, :], S[i]["Oj"][:, 0:ACW], Act.Ln,
                                     scale=SC_sb[:, 0:1])
                S[i]["LoS"] = LoS
            for i in (0, 1):
                acc = S[i]["acc"]
                LoS = S[i]["LoS"]
                t16 = S[i]["T16e"] if j % 2 == 0 else S[i]["T16o"]
                toff = j if j % 2 == 0 else j - 1
                if j == 0:
                    nc.vector.tensor_tensor(
                        acc[:, 0:POSW], LoS[:, 0:POSW],
                        t16[:, toff : toff + POSW], Alu.add)
                else:
                    V = vsb.tile([128, POSW], F16, tag=f"V{i}")
                    nc.vector.tensor_tensor(
                        V[:, :], LoS[:, 0:POSW],
                        t16[:, toff : toff + POSW], Alu.add)
                    nc.vector.tensor_tensor(
                        acc[:, 0:POSW], V[:, :], acc[:, 0:POSW], Alu.max)
        for i in (0, 1):
            acc = S[i]["acc"]
            nc.vector.tensor_scalar(out=acc[:, 0:POSW], in0=acc[:, 0:POSW],
                                    scalar1=AFL_sb[:, 0:1], scalar2=None,
                                    op0=Alu.max)
            m = msb.tile([128, HO * WO], F16, tag=f"m{i}")
            nc.scalar.activation(
                m.rearrange("q (a b) -> q a b", a=HO),
                acc.rearrange("q (a b) -> q a b", b=W)[:, :, :WO],
                Act.Exp, bias=EXM_sb[:, 0:1], scale=1.0 / T)
            m_tiles.append(m)

        # combine + transpose: pt[pos, c] = m1.T @ [I;-I] + m2.T @ [-I;I]
        m1, m2 = m_tiles
        for c0 in range(0, HO * WO, 128):
            cw = min(128, HO * WO - c0)
            pt = psO.tile([128, XW], F32, tag="Oj")
            nc.tensor.matmul(pt[:cw, 0:COUT], lhsT=m1[:, c0 : c0 + cw],
                             rhs=MM_sb[:, 0:COUT], start=True, stop=False)
            nc.tensor.matmul(pt[:cw, 0:COUT], lhsT=m2[:, c0 : c0 + cw],
                             rhs=MM_sb[:, COUT:128], start=False, stop=True)
            yt = ysb.tile([128, COUT], F32, tag="yt")
            nc.vector.tensor_tensor(yt[:cw, :], pt[:cw, 0:COUT], BC_sb[:cw, :], Alu.add)
            nc.sync.dma_start(out=Y[c0 : c0 + cw, :], in_=yt[:cw, :])
    nc.finalize()
    return nc


def _host_prep(x, k1, k2, bias):
    x = np.ascontiguousarray(np.asarray(x, dtype=np.float32))
    k1 = np.asarray(k1, np.float32).reshape(3, 3, C, COUT)
    k2 = np.asarray(k2, np.float32).reshape(3, 3, C, COUT)
    Mk1 = k1.reshape(-1, COUT).max(axis=0)
    Mk2 = k2.reshape(-1, COUT).max(axis=0)
    K3 = np.zeros((96, 384), np.float32)
    for j in range(3):
        for g in range(3):
            K3[32 * g : 32 * g + 32, 128 * j : 128 * j + 64] = \
                np.exp(T * (k1[g, j] - Mk1))
            K3[32 * g : 32 * g + 32, 128 * j + 64 : 128 * j + 128] = \
                np.exp(T * (k2[g, j] - Mk2))
    I64 = np.eye(COUT, dtype=np.float16)
    MM = np.zeros((128, 128), np.float16)
    MM[0:64, 0:COUT] = I64
    MM[64:128, 0:COUT] = -I64
    MM[0:64, COUT:128] = -I64
    MM[64:128, COUT:128] = I64
    rng1 = Mk1 - k1.reshape(-1, COUT).min(axis=0)
    rng2 = Mk2 - k2.reshape(-1, COUT).min(axis=0)
    gc = np.minimum((16.2 + T * np.concatenate([rng1, rng2]) - 5.0) / 2.0, GCAP)
    Mk = np.concatenate([Mk1, Mk2])
    EXM = (Mk + LG - (CSH + gc) / T).reshape(128, 1)
    SC = np.exp(gc).reshape(128, 1)
    AFL = (T * (np.log(0.1) - LG) + CSH + gc).reshape(128, 1)
    ONES = np.ones((96, 128), np.float32)
    BC = np.tile(np.asarray(bias, np.float32).reshape(1, COUT), (128, 1))
    shared = dict(K3=K3, MM=MM, EXM=EXM.astype(np.float32),
                  SC=SC.astype(np.float32), AFL=AFL.astype(np.float32),
                  ONES=ONES, BC=np.ascontiguousarray(BC))
    in_maps = []
    for n in range(N_CORES):
        xT = np.zeros((C, XIN), np.float32)
        xT[:, :NPIX] = x[n].reshape(NPIX, C).T
        in_maps.append({"xT": xT, **shared})
    return in_maps


def kernel(x, k1, k2, bias):
    global last_results
    if "nc" not in _cache:
        _cache["nc"] = _build_module()
    nc = _cache["nc"]
    in_maps = _host_prep(x, k1, k2, bias)
    trace = bool(int(os.environ.get("KTRACE", "0")))
    if trace:
        _ensure_axon_ntff_hook()
    res = run_bass_kernel_spmd(
        nc, in_maps, core_ids=list(range(N_CORES)), trace=trace,
    )
    last_results = res
    y = np.stack([r["Y"].reshape(HO, WO, COUT) for r in res.results], axis=0)
    return y.astype(np.float32)


# revision 22
# speedup vs baseline: 7.9484x; 1.0685x over previous
"""Bipolar morphological conv2d kernel for Trainium2 (8 NeuronCores).

Math: reference computes, per output position and out-channel c,
    y = m(lp1,K1) - m(lp1,K2) - m(lp2,K1) + m(lp2,K2) + bias
with m(logp, k)[c] = exp(max_p(logp_p + k_pc)), lp1 = log(max(x, .1)),
lp2 = log(max(-x, .1)).

Device algorithm (data-parallel, one batch image per core): the 288-tap
max-plus reduction is evaluated as a tight LSE (p-norm, t=112) over each
3x1 column group of the 3x3 window (96 entries: 3 rows x 32 channels),
turning the heavy reduction into THREE K=96 TensorE matmuls, followed by
an EXACT max over the 3 column groups in log domain (DVE fp16).  A
per-rhs-column normalizer M3q (itself a sigma=32 LSE, computed with a
ones-matmul) keeps every exponential in fp32 range; its value cancels
exactly in the algebra, so only over/underflow matters, not its accuracy.

Layout: channel-major pixel rows.  xT3 [96, 1024] holds the 3 row-shifted
copies of xT [32, 1024] (partition 32g+ci = channel ci shifted g rows),
so a column-group matmul contracts all 96 entries in one instruction and
tap shifts become free-dim column offsets (multiples of 1).  Out-channels
of K1|K2 are stacked on the 128 output partitions, so one matmul chain
serves both kernels.  Final exp folds the per-channel max-k and the
30x30 window selection; a pair of +/-I matmuls transposes to
position-major and combines the 4 morphs with their signs.
"""

import os
from contextlib import ExitStack

import numpy as np

import concourse.bass as bass
import concourse.mybir as mybir
from concourse import bacc
import concourse.tile as tile
from concourse.bass_utils import run_bass_kernel_spmd
from concourse.hw_specs import get_activation_tables
import bass_rust as _bass_rust


class _BaccOneActSet(bacc.Bacc):
    """Bacc whose act-table pass may only pick natural_log_exp_and_others
    (contains Ln+Exp+Copy, the only functions used here), so the table is
    loaded once instead of thrashing on every Ln<->Exp transition."""

    def insert_act_table_loads(self):
        has_activation = any(
            isinstance(i, mybir.InstActivation)
            for b in self.main_func.blocks
            for i in b.instructions
        )
        if not has_activation:
            return
        tables = [
            (n, (fns if n == "natural_log_exp_and_others" else set()))
            for n, fns in get_activation_tables(self.m.arch).items()
        ]
        _bass_rust.insert_act_table_loads(self, tables)

N_CORES = 8
H = W = C = 32
COUT = 64
HO = WO = 30
NPIX = H * W            # 1024
XW = 1024               # working row width (pixels)
XIN = 1088              # input row width (1024 + 64 pad for row shifts)
ACW = 960               # accumulator width (30 rows x 32 cols)
POSW = 958              # last used pos col is 29*32+29 = 957

SIG1 = 8.0              # stage-1 normalizer LSE sharpness (fits ACT Ln window)
SIG2 = 32.0             # stage-2 normalizer refinement sharpness
T = 112.0               # main LSE sharpness
G = 6.0                 # fixed global scale bound (|x| < 6 for N(0,1) data)
LG = float(np.log(G))
CSH = 216.0             # fp16 recentering shift for the log-domain combine
GCAP = 38.5             # cap on per-channel ln-rescale (ACT Ln window ~|44|)

F32 = mybir.dt.float32
F16 = mybir.dt.float16
_cache: dict = {}
last_results = None


def _ensure_axon_ntff_hook():
    """The trimmed agent image lacks antenv.axon_hooks; recreate it so
    run_bass_kernel_spmd(trace=True) can capture NTFF profiles. No-op on
    failure (tracing then just degrades)."""
    import sys
    import types

    try:
        import antenv.axon_hooks  # noqa: F401
        return
    except ImportError:
        pass
    try:
        mod = types.ModuleType("antenv.axon_hooks")
        holder = [None]
        mod.set_axon_ntff_profile_hook = lambda h: holder.__setitem__(0, h)
        mod.get_axon_ntff_profile_hook = lambda: holder[0]
        sys.modules["antenv.axon_hooks"] = mod
        from trn_agent_boot.trn_boot import _ntff_profile_via_ctypes

        so = "/opt/axon/libaxon_pjrt.so"
        if os.path.exists(so):
            holder[0] = _ntff_profile_via_ctypes(so)
    except Exception:
        pass


def _build_module():
    nc = _BaccOneActSet()
    Alu = mybir.AluOpType
    Act = mybir.ActivationFunctionType

    xT = nc.dram_tensor("xT", [C, XIN], F32, kind="ExternalInput")
    K3 = nc.dram_tensor("K3", [96, 384], F32, kind="ExternalInput")
    MM = nc.dram_tensor("MM", [128, 128], F16, kind="ExternalInput")
    CM = nc.dram_tensor("CM", [128, 3 + COUT], F32, kind="ExternalInput")
    Y = nc.dram_tensor("Y", [HO * WO, COUT], F32, kind="ExternalOutput")

    with tile.TileContext(nc) as tc, ExitStack() as ctx:
        const = ctx.enter_context(tc.tile_pool(name="const", bufs=1))
        sp = ctx.enter_context(tc.tile_pool(name="sp", bufs=1))
        losb = ctx.enter_context(tc.tile_pool(name="losb", bufs=3))
        accp = ctx.enter_context(tc.tile_pool(name="accp", bufs=1))
        vsb = ctx.enter_context(tc.tile_pool(name="vsb", bufs=1))
        msb = ctx.enter_context(tc.tile_pool(name="msb", bufs=1))
        ysb = ctx.enter_context(tc.tile_pool(name="ysb", bufs=2))
        psS = ctx.enter_context(tc.tile_pool(name="psS", bufs=2, space="PSUM"))
        psO = ctx.enter_context(tc.tile_pool(name="psO", bufs=2, space="PSUM"))

        # ---- constants / input staging ----
        xT3 = const.tile([96, XW], F32)
        for g in range(3):
            nc.gpsimd.dma_start(out=xT3[32 * g : 32 * g + 32, :],
                                in_=xT[:, 32 * g : 32 * g + XW])
        K3_sb = const.tile([96, 384], F32)
        nc.sync.dma_start(out=K3_sb[:, :], in_=K3[:, :])
        MM_sb = const.tile([128, 128], F16)
        nc.sync.dma_start(out=MM_sb[:, :], in_=MM[:, :])
        CM_sb = const.tile([128, 3 + COUT], F32)
        nc.sync.dma_start(out=CM_sb[:, :], in_=CM[:, :])
        EXM_sb, SC_sb, AFL_sb = CM_sb[:, 0:1], CM_sb[:, 1:2], CM_sb[:, 2:3]
        BC_sb = CM_sb[:, 3 : 3 + COUT]
        ONES_sb = const.tile([96, 128], F32)
        nc.gpsimd.memset(ONES_sb[:, :], 1.0)
        B8_sb = const.tile([96, 1], F32)
        nc.gpsimd.memset(B8_sb[:, :], -SIG1 * LG)
        B32_sb = const.tile([96, 1], F32)
        nc.gpsimd.memset(B32_sb[:, :], -SIG2 * LG)
        B96_sb = const.tile([96, 1], F32)
        nc.gpsimd.memset(B96_sb[:, :], -T * LG)

        m_tiles = []
        S = [dict(), dict()]
        for i, sgn in ((0, 1.0), (1, -1.0)):
            X3 = sp.tile([96, XW], F32, tag=f"X3{i}")
            nc.vector.tensor_scalar(out=X3[:, :], in0=xT3[:, :],
                                    scalar1=sgn, scalar2=0.1,
                                    op0=Alu.mult, op1=Alu.max)
            S[i]["X3"] = X3
        for i in (0, 1):
            lp3 = sp.tile([96, XW], F32, tag=f"lp3{i}")
            nc.scalar.activation(lp3[:, :], S[i]["X3"][:, :], Act.Ln)
            S[i]["lp3"] = lp3
        for i in (0, 1):
            E8 = sp.tile([96, XW], F32, tag=f"E8{i}")
            nc.scalar.activation(E8[:, :], S[i]["lp3"][:, :], Act.Exp,
                                 bias=B8_sb[:, 0:1], scale=SIG1)
            S[i]["E8"] = E8
        for i in (0, 1):
            S8p = psS.tile([128, XW], F32, tag="Sp")
            for c0 in (0, 512):
                nc.tensor.matmul(S8p[:, c0 : c0 + 512], lhsT=ONES_sb[:, :],
                                 rhs=S[i]["E8"][:, c0 : c0 + 512],
                                 start=True, stop=True)
            S[i]["S8p"] = S8p
        for i in (0, 1):
            L8 = sp.tile([128, XW], F32, tag=f"L8{i}")
            nc.scalar.activation(L8[:, :], S[i]["S8p"][:, :], Act.Ln)
            S[i]["L8"] = L8
        for i in (0, 1):
            d8 = sp.tile([96, XW], F32, tag=f"d8{i}")
            nc.vector.scalar_tensor_tensor(out=d8[:, :], in0=S[i]["L8"][0:96, :],
                                           scalar=-1.0 / SIG1, in1=S[i]["lp3"][:, :],
                                           op0=Alu.mult, op1=Alu.add)
            S[i]["d8"] = d8
        for i in (0, 1):
            E32 = sp.tile([96, XW], F32, tag=f"E32{i}")
            nc.scalar.activation(E32[:, :], S[i]["d8"][:, :], Act.Exp,
                                 bias=B32_sb[:, 0:1], scale=SIG2)
            S[i]["E32"] = E32
        for i in (0, 1):
            S32p = psS.tile([128, XW], F32, tag="Sp")
            for c0 in (0, 512):
                nc.tensor.matmul(S32p[:, c0 : c0 + 512], lhsT=ONES_sb[:, :],
                                 rhs=S[i]["E32"][:, c0 : c0 + 512],
                                 start=True, stop=True)
            S[i]["S32p"] = S32p
        for i in (0, 1):
            L32 = sp.tile([128, XW], F32, tag=f"L32{i}")
            nc.scalar.activation(L32[:, :], S[i]["S32p"][:, :], Act.Ln)
            S[i]["L32"] = L32
        for i in (0, 1):
            LS0 = sp.tile([128, XW], F32, tag=f"LS0{i}")
            nc.vector.scalar_tensor_tensor(out=LS0[:, :], in0=S[i]["L8"][:, :],
                                           scalar=SIG2 / SIG1, in1=S[i]["L32"][:, :],
                                           op0=Alu.mult, op1=Alu.add)
            S[i]["LS0"] = LS0
        for i in (0, 1):
            d96 = sp.tile([96, XW], F32, tag=f"d96{i}")
            nc.vector.scalar_tensor_tensor(out=d96[:, :], in0=S[i]["LS0"][0:96, :],
                                           scalar=-1.0 / SIG2, in1=S[i]["lp3"][:, :],
                                           op0=Alu.mult, op1=Alu.add)
            S[i]["d96"] = d96
        for i in (0, 1):
            E96 = sp.tile([96, XW], F32, tag=f"E96{i}")
            nc.scalar.activation(E96[:, :], S[i]["d96"][:, :], Act.Exp,
                                 bias=B96_sb[:, 0:1], scale=T)
            S[i]["E96"] = E96
        for i in (0, 1):
            T16e = sp.tile([128, XW], F16, tag=f"T16e{i}")
            nc.vector.tensor_scalar(out=T16e[:, :], in0=S[i]["LS0"][:, :],
                                    scalar1=T / SIG2, scalar2=CSH,
                                    op0=Alu.mult, op1=Alu.add)
            T16o = sp.tile([128, XW - 2], F16, tag=f"T16o{i}")
            nc.vector.tensor_scalar(out=T16o[:, :], in0=S[i]["LS0"][:, 1 : XW - 1],
                                    scalar1=T / SIG2, scalar2=CSH,
                                    op0=Alu.mult, op1=Alu.add)
            S[i]["T16e"], S[i]["T16o"] = T16e, T16o
            acc = accp.tile([128, ACW], F16, tag=f"acc{i}")
            S[i]["acc"] = acc
        for j in range(3):
            for i in (0, 1):
                Oj = psO.tile([128, XW], F32, tag="Oj")
                E96 = S[i]["E96"]
                nc.tensor.matmul(Oj[:, 0:512], lhsT=K3_sb[:, 128 * j : 128 * j + 128],
                                 rhs=E96[:, j : j + 512], start=True, stop=True)
                nc.tensor.matmul(Oj[:, 512:ACW], lhsT=K3_sb[:, 128 * j : 128 * j + 128],
                                 rhs=E96[:, j + 512 : j + ACW], start=True, stop=True)
                S[i]["Oj"] = Oj
            for i in (0, 1):
                LoS = losb.tile([128, ACW], F16, tag=f"LoS{i}")
                nc.scalar.activation(LoS[:, :], S[i]["Oj"][:, 0:ACW], Act.Ln,
                                     scale=SC_sb)
                S[i]["LoS"] = LoS
            for i in (0, 1):
                acc = S[i]["acc"]
                LoS = S[i]["LoS"]
                t16 = S[i]["T16e"] if j % 2 == 0 else S[i]["T16o"]
                toff = j if j % 2 == 0 else j - 1
                if j == 0:
                    nc.vector.tensor_tensor(
                        acc[:, 0:POSW], LoS[:, 0:POSW],
                        t16[:, toff : toff + POSW], Alu.add)
                else:
                    V = vsb.tile([128, POSW], F16, tag=f"V{i}")
                    nc.vector.tensor_tensor(
                        V[:, :], LoS[:, 0:POSW],
                        t16[:, toff : toff + POSW], Alu.add)
                    nc.vector.tensor_tensor(
                        acc[:, 0:POSW], V[:, :], acc[:, 0:POSW], Alu.max)
        for i in (0, 1):
            acc = S[i]["acc"]
            nc.vector.tensor_scalar(out=acc[:, 0:POSW], in0=acc[:, 0:POSW],
                                    scalar1=AFL_sb, scalar2=None,
                                    op0=Alu.max)
            m = msb.tile([128, HO * WO], F16, tag=f"m{i}")
            nc.scalar.activation(
                m.rearrange("q (a b) -> q a b", a=HO),
                acc.rearrange("q (a b) -> q a b", b=W)[:, :, :WO],
                Act.Exp, bias=EXM_sb, scale=1.0 / T)
            m_tiles.append(m)

        # combine + transpose: pt[pos, c] = m1.T @ [I;-I] + m2.T @ [-I;I]
        m1, m2 = m_tiles
        for c0 in range(0, HO * WO, 128):
            cw = min(128, HO * WO - c0)
            pt = psO.tile([128, XW], F32, tag="Oj")
            nc.tensor.matmul(pt[:cw, 0:COUT], lhsT=m1[:, c0 : c0 + cw],
                             rhs=MM_sb[:, 0:COUT], start=True, stop=False)
            nc.tensor.matmul(pt[:cw, 0:COUT], lhsT=m2[:, c0 : c0 + cw],
                             rhs=MM_sb[:, COUT:128], start=False, stop=True)
            yt = ysb.tile([128, COUT], F32, tag="yt")
            nc.vector.tensor_tensor(yt[:cw, :], pt[:cw, 0:COUT], BC_sb[:cw], Alu.add)
            nc.sync.dma_start(out=Y[c0 : c0 + cw, :], in_=yt[:cw, :])
    nc.finalize()
    return nc


def _host_prep(x, k1, k2, bias):
    x = np.ascontiguousarray(np.asarray(x, dtype=np.float32))
    k1 = np.asarray(k1, np.float32).reshape(3, 3, C, COUT)
    k2 = np.asarray(k2, np.float32).reshape(3, 3, C, COUT)
    Mk1 = k1.reshape(-1, COUT).max(axis=0)
    Mk2 = k2.reshape(-1, COUT).max(axis=0)
    K3 = np.zeros((96, 384), np.float32)
    for j in range(3):
        for g in range(3):
            K3[32 * g : 32 * g + 32, 128 * j : 128 * j + 64] = \
                np.exp(T * (k1[g, j] - Mk1))
            K3[32 * g : 32 * g + 32, 128 * j + 64 : 128 * j + 128] = \
                np.exp(T * (k2[g, j] - Mk2))
    I64 = np.eye(COUT, dtype=np.float16)
    MM = np.zeros((128, 128), np.float16)
    MM[0:64, 0:COUT] = I64
    MM[64:128, 0:COUT] = -I64
    MM[0:64, COUT:128] = -I64
    MM[64:128, COUT:128] = I64
    rng1 = Mk1 - k1.reshape(-1, COUT).min(axis=0)
    rng2 = Mk2 - k2.reshape(-1, COUT).min(axis=0)
    gc = np.minimum((16.2 + T * np.concatenate([rng1, rng2]) - 5.0) / 2.0, GCAP)
    Mk = np.concatenate([Mk1, Mk2])
    CM = np.zeros((128, 3 + COUT), np.float32)
    CM[:, 0] = Mk + LG - (CSH + gc) / T
    CM[:, 1] = np.exp(gc)
    CM[:, 2] = T * (np.log(0.1) - LG) + CSH + gc
    CM[:, 3:] = np.asarray(bias, np.float32).reshape(1, COUT)
    shared = dict(K3=K3, MM=MM, CM=CM)
    in_maps = []
    for n in range(N_CORES):
        xT = np.zeros((C, XIN), np.float32)
        xT[:, :NPIX] = x[n].reshape(NPIX, C).T
        in_maps.append({"xT": xT, **shared})
    return in_maps


def kernel(x, k1, k2, bias):
    global last_results
    if "nc" not in _cache:
        _cache["nc"] = _build_module()
    nc = _cache["nc"]
    in_maps = _host_prep(x, k1, k2, bias)
    trace = bool(int(os.environ.get("KTRACE", "0")))
    if trace:
        _ensure_axon_ntff_hook()
    res = run_bass_kernel_spmd(
        nc, in_maps, core_ids=list(range(N_CORES)), trace=trace,
    )
    last_results = res
    y = np.stack([r["Y"].reshape(HO, WO, COUT) for r in res.results], axis=0)
    return y.astype(np.float32)


# revision 25
# speedup vs baseline: 9.0667x; 1.1407x over previous
"""Bipolar morphological conv2d kernel for Trainium2 (8 NeuronCores).

Math: reference computes, per output position and out-channel c,
    y = m(lp1,K1) - m(lp1,K2) - m(lp2,K1) + m(lp2,K2) + bias
with m(logp, k)[c] = exp(max_p(logp_p + k_pc)), lp1 = log(max(x, .1)),
lp2 = log(max(-x, .1)).

Device algorithm (data-parallel, one batch image per core): the 288-tap
max-plus reduction is evaluated as a tight LSE (p-norm, t=112) over each
3x1 column group of the 3x3 window (96 entries: 3 rows x 32 channels),
turning the heavy reduction into THREE K=96 TensorE matmuls, followed by
an EXACT max over the 3 column groups in log domain (DVE fp16).  A
per-rhs-column normalizer M3q (itself a sigma=32 LSE, computed with a
ones-matmul) keeps every exponential in fp32 range; its value cancels
exactly in the algebra, so only over/underflow matters, not its accuracy.

Layout: channel-major pixel rows.  xT3 [96, 1024] holds the 3 row-shifted
copies of xT [32, 1024] (partition 32g+ci = channel ci shifted g rows),
so a column-group matmul contracts all 96 entries in one instruction and
tap shifts become free-dim column offsets (multiples of 1).  Out-channels
of K1|K2 are stacked on the 128 output partitions, so one matmul chain
serves both kernels.  Final exp folds the per-channel max-k and the
30x30 window selection; a pair of +/-I matmuls transposes to
position-major and combines the 4 morphs with their signs.
"""

import os
from contextlib import ExitStack

import numpy as np

import concourse.bass as bass
import concourse.mybir as mybir
from concourse import bacc
import concourse.tile as tile
from concourse.bass_utils import run_bass_kernel_spmd
from concourse.hw_specs import get_activation_tables
import bass_rust as _bass_rust


class _BaccOneActSet(bacc.Bacc):
    """Bacc whose act-table pass may only pick natural_log_exp_and_others
    (contains Ln+Exp+Copy, the only functions used here), so the table is
    loaded once instead of thrashing on every Ln<->Exp transition."""

    def insert_act_table_loads(self):
        has_activation = any(
            isinstance(i, mybir.InstActivation)
            for b in self.main_func.blocks
            for i in b.instructions
        )
        if not has_activation:
            return
        tables = [
            (n, (fns if n == "natural_log_exp_and_others" else set()))
            for n, fns in get_activation_tables(self.m.arch).items()
        ]
        _bass_rust.insert_act_table_loads(self, tables)

N_CORES = 8
H = W = C = 32
COUT = 64
HO = WO = 30
NPIX = H * W            # 1024
XW = 1024               # working row width (pixels)
XIN = 1088              # input row width (1024 + 64 pad for row shifts)
ACW = 960               # accumulator width (30 rows x 32 cols)
POSW = 958              # last used pos col is 29*32+29 = 957

SIG1 = 8.0              # stage-1 normalizer LSE sharpness (fits ACT Ln window)
SIG2 = 32.0             # stage-2 normalizer refinement sharpness
T = 112.0               # main LSE sharpness
G = 6.0                 # fixed global scale bound (|x| < 6 for N(0,1) data)
LG = float(np.log(G))
CSH = 216.0             # fp16 recentering shift for the log-domain combine
GCAP = 38.5             # cap on per-channel ln-rescale (ACT Ln window ~|44|)

F32 = mybir.dt.float32
F16 = mybir.dt.float16
BF16 = mybir.dt.bfloat16
_cache: dict = {}
last_results = None


def _ensure_axon_ntff_hook():
    """The trimmed agent image lacks antenv.axon_hooks; recreate it so
    run_bass_kernel_spmd(trace=True) can capture NTFF profiles. No-op on
    failure (tracing then just degrades)."""
    import sys
    import types

    try:
        import antenv.axon_hooks  # noqa: F401
        return
    except ImportError:
        pass
    try:
        mod = types.ModuleType("antenv.axon_hooks")
        holder = [None]
        mod.set_axon_ntff_profile_hook = lambda h: holder.__setitem__(0, h)
        mod.get_axon_ntff_profile_hook = lambda: holder[0]
        sys.modules["antenv.axon_hooks"] = mod
        from trn_agent_boot.trn_boot import _ntff_profile_via_ctypes

        so = "/opt/axon/libaxon_pjrt.so"
        if os.path.exists(so):
            holder[0] = _ntff_profile_via_ctypes(so)
    except Exception:
        pass


def _build_module():
    nc = _BaccOneActSet()
    Alu = mybir.AluOpType
    Act = mybir.ActivationFunctionType

    xT = nc.dram_tensor("xT", [C, XIN], F32, kind="ExternalInput")
    K3 = nc.dram_tensor("K3", [96, 384], BF16, kind="ExternalInput")
    MM = nc.dram_tensor("MM", [128, 128], F16, kind="ExternalInput")
    CM = nc.dram_tensor("CM", [128, 3 + COUT], F32, kind="ExternalInput")
    Y = nc.dram_tensor("Y", [HO * WO, COUT], F32, kind="ExternalOutput")

    with tile.TileContext(nc) as tc, ExitStack() as ctx:
        const = ctx.enter_context(tc.tile_pool(name="const", bufs=1))
        sp = ctx.enter_context(tc.tile_pool(name="sp", bufs=1))
        losb = ctx.enter_context(tc.tile_pool(name="losb", bufs=3))
        accp = ctx.enter_context(tc.tile_pool(name="accp", bufs=1))
        vsb = ctx.enter_context(tc.tile_pool(name="vsb", bufs=1))
        msb = ctx.enter_context(tc.tile_pool(name="msb", bufs=1))
        ysb = ctx.enter_context(tc.tile_pool(name="ysb", bufs=2))
        psS = ctx.enter_context(tc.tile_pool(name="psS", bufs=2, space="PSUM"))
        psO = ctx.enter_context(tc.tile_pool(name="psO", bufs=2, space="PSUM"))

        # ---- constants / input staging ----
        xT3 = const.tile([96, XW], F32)
        for g in range(3):
            nc.gpsimd.dma_start(out=xT3[32 * g : 32 * g + 32, :],
                                in_=xT[:, 32 * g : 32 * g + XW])
        K3_sb = const.tile([96, 384], BF16)
        nc.sync.dma_start(out=K3_sb[:, :], in_=K3[:, :])
        MM_sb = const.tile([128, 128], F16)
        nc.sync.dma_start(out=MM_sb[:, :], in_=MM[:, :])
        CM_sb = const.tile([128, 3 + COUT], F32)
        nc.sync.dma_start(out=CM_sb[:, :], in_=CM[:, :])
        EXM_sb, SC_sb, AFL_sb = CM_sb[:, 0:1], CM_sb[:, 1:2], CM_sb[:, 2:3]
        BC_sb = CM_sb[:, 3 : 3 + COUT]
        ONES_sb = const.tile([96, 128], BF16)
        nc.gpsimd.memset(ONES_sb[:, :], 1.0)
        B8_sb = const.tile([96, 1], F32)
        nc.gpsimd.memset(B8_sb[:, :], -SIG1 * LG)
        B32_sb = const.tile([96, 1], F32)
        nc.gpsimd.memset(B32_sb[:, :], -SIG2 * LG)
        B96_sb = const.tile([96, 1], F32)
        nc.gpsimd.memset(B96_sb[:, :], -T * LG)

        m_tiles = []
        S = [dict(), dict()]
        for i, sgn in ((0, 1.0), (1, -1.0)):
            X3 = sp.tile([96, XW], F32, tag=f"X3{i}")
            nc.vector.tensor_scalar(out=X3[:, :], in0=xT3[:, :],
                                    scalar1=sgn, scalar2=0.1,
                                    op0=Alu.mult, op1=Alu.max)
            S[i]["X3"] = X3
        for i in (0, 1):
            lp3 = sp.tile([96, XW], F32, tag=f"lp3{i}")
            nc.scalar.activation(lp3[:, :], S[i]["X3"][:, :], Act.Ln)
            S[i]["lp3"] = lp3
        for i in (0, 1):
            E8 = sp.tile([96, XW], BF16, tag=f"E8{i}")
            nc.scalar.activation(E8[:, :], S[i]["lp3"][:, :], Act.Exp,
                                 bias=B8_sb[:, 0:1], scale=SIG1)
            S[i]["E8"] = E8
        for i in (0, 1):
            S8p = psS.tile([128, XW], F32, tag="Sp")
            for c0 in (0, 512):
                nc.tensor.matmul(S8p[:, c0 : c0 + 512], lhsT=ONES_sb[:, :],
                                 rhs=S[i]["E8"][:, c0 : c0 + 512],
                                 start=True, stop=True)
            S[i]["S8p"] = S8p
        for i in (0, 1):
            L8 = sp.tile([128, XW], F32, tag=f"L8{i}")
            nc.scalar.activation(L8[:, :], S[i]["S8p"][:, :], Act.Ln)
            S[i]["L8"] = L8
        for i in (0, 1):
            d8 = sp.tile([96, XW], F32, tag=f"d8{i}")
            nc.vector.scalar_tensor_tensor(out=d8[:, :], in0=S[i]["L8"][0:96, :],
                                           scalar=-1.0 / SIG1, in1=S[i]["lp3"][:, :],
                                           op0=Alu.mult, op1=Alu.add)
            S[i]["d8"] = d8
        for i in (0, 1):
            E32 = sp.tile([96, XW], BF16, tag=f"E32{i}")
            nc.scalar.activation(E32[:, :], S[i]["d8"][:, :], Act.Exp,
                                 bias=B32_sb[:, 0:1], scale=SIG2)
            S[i]["E32"] = E32
        for i in (0, 1):
            S32p = psS.tile([128, XW], F32, tag="Sp")
            for c0 in (0, 512):
                nc.tensor.matmul(S32p[:, c0 : c0 + 512], lhsT=ONES_sb[:, :],
                                 rhs=S[i]["E32"][:, c0 : c0 + 512],
                                 start=True, stop=True)
            S[i]["S32p"] = S32p
        for i in (0, 1):
            L32 = sp.tile([128, XW], F32, tag=f"L32{i}")
            nc.scalar.activation(L32[:, :], S[i]["S32p"][:, :], Act.Ln)
            S[i]["L32"] = L32
        for i in (0, 1):
            # d96 = lp - M3q + lG = d8 - L32/SIG2  (critical path, DVE)
            d96 = sp.tile([96, XW], F32, tag=f"d96{i}")
            nc.vector.scalar_tensor_tensor(out=d96[:, :], in0=S[i]["L32"][0:96, :],
                                           scalar=-1.0 / SIG2, in1=S[i]["d8"][:, :],
                                           op0=Alu.mult, op1=Alu.add)
            S[i]["d96"] = d96
        for i in (0, 1):
            E96 = sp.tile([96, XW], BF16, tag=f"E96{i}")
            nc.scalar.activation(E96[:, :], S[i]["d96"][:, :], Act.Exp,
                                 bias=B96_sb[:, 0:1], scale=T)
            S[i]["E96"] = E96
        for i in (0, 1):
            # T16 = (T/SIG1)*L8 + (T/SIG2)*L32 + CSH, off critical path (GpSimd)
            LS0 = sp.tile([128, XW], F32, tag=f"LS0{i}")
            nc.vector.scalar_tensor_tensor(out=LS0[:, :], in0=S[i]["L8"][:, :],
                                           scalar=SIG2 / SIG1, in1=S[i]["L32"][:, :],
                                           op0=Alu.mult, op1=Alu.add)
            T16e = sp.tile([128, XW], F16, tag=f"T16e{i}")
            nc.vector.tensor_scalar(out=T16e[:, :], in0=LS0[:, :],
                                    scalar1=T / SIG2, scalar2=CSH,
                                    op0=Alu.mult, op1=Alu.add)
            S[i]["T16e"] = T16e
            acc = accp.tile([128, ACW], F16, tag=f"acc{i}")
            S[i]["acc"] = acc
        for j in range(3):
            for i in (0, 1):
                Oj = psO.tile([128, XW], F32, tag="Oj")
                E96 = S[i]["E96"]
                nc.tensor.matmul(Oj[:, 0:512], lhsT=K3_sb[:, 128 * j : 128 * j + 128],
                                 rhs=E96[:, j : j + 512], start=True, stop=True)
                nc.tensor.matmul(Oj[:, 512:ACW], lhsT=K3_sb[:, 128 * j : 128 * j + 128],
                                 rhs=E96[:, j + 512 : j + ACW], start=True, stop=True)
                S[i]["Oj"] = Oj
            for i in (0, 1):
                LoS = losb.tile([128, ACW], F16, tag=f"LoS{i}")
                nc.scalar.activation(LoS[:, :], S[i]["Oj"][:, 0:ACW], Act.Ln,
                                     scale=SC_sb)
                S[i]["LoS"] = LoS
            for i in (0, 1):
                acc = S[i]["acc"]
                LoS = S[i]["LoS"]
                t16 = S[i]["T16e"]
                toff = j
                if j == 0:
                    nc.vector.tensor_tensor(
                        acc[:, 0:POSW], LoS[:, 0:POSW],
                        t16[:, toff : toff + POSW], Alu.add)
                else:
                    V = vsb.tile([128, POSW], F16, tag=f"V{i}")
                    nc.vector.tensor_tensor(
                        V[:, :], LoS[:, 0:POSW],
                        t16[:, toff : toff + POSW], Alu.add)
                    nc.vector.tensor_tensor(
                        acc[:, 0:POSW], V[:, :], acc[:, 0:POSW], Alu.max)
        for i in (0, 1):
            acc = S[i]["acc"]
            nc.vector.tensor_scalar(out=acc[:, 0:POSW], in0=acc[:, 0:POSW],
                                    scalar1=AFL_sb, scalar2=None,
                                    op0=Alu.max)
            m = msb.tile([128, HO * WO], F16, tag=f"m{i}")
            nc.scalar.activation(
                m.rearrange("q (a b) -> q a b", a=HO),
                acc.rearrange("q (a b) -> q a b", b=W)[:, :, :WO],
                Act.Exp, bias=EXM_sb, scale=1.0 / T)
            m_tiles.append(m)

        # combine + transpose: pt[pos, c] = m1.T @ [I;-I] + m2.T @ [-I;I]
        m1, m2 = m_tiles
        for c0 in range(0, HO * WO, 128):
            cw = min(128, HO * WO - c0)
            pt = psS.tile([128, XW], F32, tag="Sp")
            nc.tensor.matmul(pt[:cw, 0:COUT], lhsT=m1[:, c0 : c0 + cw],
                             rhs=MM_sb[:, 0:COUT], start=True, stop=False)
            nc.tensor.matmul(pt[:cw, 0:COUT], lhsT=m2[:, c0 : c0 + cw],
                             rhs=MM_sb[:, COUT:128], start=False, stop=True)
            yt = ysb.tile([128, COUT], F32, tag="yt")
            nc.vector.tensor_tensor(yt[:cw, :], pt[:cw, 0:COUT], BC_sb[:cw], Alu.add)
            nc.sync.dma_start(out=Y[c0 : c0 + cw, :], in_=yt[:cw, :])
    nc.finalize()
    return nc


def _host_prep(x, k1, k2, bias):
    x = np.ascontiguousarray(np.asarray(x, dtype=np.float32))
    k1 = np.asarray(k1, np.float32).reshape(3, 3, C, COUT)
    k2 = np.asarray(k2, np.float32).reshape(3, 3, C, COUT)
    Mk1 = k1.reshape(-1, COUT).max(axis=0)
    Mk2 = k2.reshape(-1, COUT).max(axis=0)
    K3 = np.zeros((96, 384), np.float32)  # cast to bf16 below
    for j in range(3):
        for g in range(3):
            K3[32 * g : 32 * g + 32, 128 * j : 128 * j + 64] = \
                np.exp(T * (k1[g, j] - Mk1))
            K3[32 * g : 32 * g + 32, 128 * j + 64 : 128 * j + 128] = \
                np.exp(T * (k2[g, j] - Mk2))
    I64 = np.eye(COUT, dtype=np.float16)
    MM = np.zeros((128, 128), np.float16)
    MM[0:64, 0:COUT] = I64
    MM[64:128, 0:COUT] = -I64
    MM[0:64, COUT:128] = -I64
    MM[64:128, COUT:128] = I64
    rng1 = Mk1 - k1.reshape(-1, COUT).min(axis=0)
    rng2 = Mk2 - k2.reshape(-1, COUT).min(axis=0)
    gc = np.minimum((16.2 + T * np.concatenate([rng1, rng2]) - 5.0) / 2.0, GCAP)
    Mk = np.concatenate([Mk1, Mk2])
    CM = np.zeros((128, 3 + COUT), np.float32)
    CM[:, 0] = Mk + LG - (CSH + gc) / T
    CM[:, 1] = np.exp(gc)
    CM[:, 2] = T * (np.log(0.1) - LG) + CSH + gc
    CM[:, 3:] = np.asarray(bias, np.float32).reshape(1, COUT)
    K3bf = (K3.view(np.uint32) >> 16).astype(np.uint16)  # truncate-to-bf16
    import ml_dtypes
    K3bf = K3.astype(ml_dtypes.bfloat16)
    shared = dict(K3=K3bf, MM=MM, CM=CM)
    in_maps = []
    for n in range(N_CORES):
        xT = np.zeros((C, XIN), np.float32)
        xT[:, :NPIX] = x[n].reshape(NPIX, C).T
        in_maps.append({"xT": xT, **shared})
    return in_maps


def kernel(x, k1, k2, bias):
    global last_results
    if "nc" not in _cache:
        _cache["nc"] = _build_module()
    nc = _cache["nc"]
    in_maps = _host_prep(x, k1, k2, bias)
    trace = bool(int(os.environ.get("KTRACE", "0")))
    if trace:
        _ensure_axon_ntff_hook()
    res = run_bass_kernel_spmd(
        nc, in_maps, core_ids=list(range(N_CORES)), trace=trace,
    )
    last_results = res
    y = np.stack([r["Y"].reshape(HO, WO, COUT) for r in res.results], axis=0)
    return y.astype(np.float32)


# revision 28
# speedup vs baseline: 9.5958x; 1.0584x over previous
"""Bipolar morphological conv2d kernel for Trainium2 (8 NeuronCores).

Math: reference computes, per output position and out-channel c,
    y = m(lp1,K1) - m(lp1,K2) - m(lp2,K1) + m(lp2,K2) + bias
with m(logp, k)[c] = exp(max_p(logp_p + k_pc)), lp1 = log(max(x, .1)),
lp2 = log(max(-x, .1)).

Device algorithm (data-parallel, one batch image per core): the 288-tap
max-plus reduction is evaluated as a tight LSE (p-norm, t=112) over each
3x1 column group of the 3x3 window (96 entries: 3 rows x 32 channels),
turning the heavy reduction into THREE K=96 TensorE matmuls, followed by
an EXACT max over the 3 column groups in log domain (DVE fp16).  A
per-rhs-column normalizer M3q (itself a sigma=32 LSE, computed with a
ones-matmul) keeps every exponential in fp32 range; its value cancels
exactly in the algebra, so only over/underflow matters, not its accuracy.

Layout: channel-major pixel rows.  xT3 [96, 1024] holds the 3 row-shifted
copies of xT [32, 1024] (partition 32g+ci = channel ci shifted g rows),
so a column-group matmul contracts all 96 entries in one instruction and
tap shifts become free-dim column offsets (multiples of 1).  Out-channels
of K1|K2 are stacked on the 128 output partitions, so one matmul chain
serves both kernels.  Final exp folds the per-channel max-k and the
30x30 window selection; a pair of +/-I matmuls transposes to
position-major and combines the 4 morphs with their signs.
"""

import os
from contextlib import ExitStack

import numpy as np

import concourse.bass as bass
import concourse.mybir as mybir
from concourse import bacc
import concourse.tile as tile
from concourse.bass_utils import run_bass_kernel_spmd
from concourse.hw_specs import get_activation_tables
import bass_rust as _bass_rust


class _BaccOneActSet(bacc.Bacc):
    """Bacc whose act-table pass may only pick natural_log_exp_and_others
    (contains Ln+Exp+Copy, the only functions used here), so the table is
    loaded once instead of thrashing on every Ln<->Exp transition."""

    def insert_act_table_loads(self):
        has_activation = any(
            isinstance(i, mybir.InstActivation)
            for b in self.main_func.blocks
            for i in b.instructions
        )
        if not has_activation:
            return
        tables = [
            (n, (fns if n == "natural_log_exp_and_others" else set()))
            for n, fns in get_activation_tables(self.m.arch).items()
        ]
        _bass_rust.insert_act_table_loads(self, tables)

N_CORES = 8
H = W = C = 32
COUT = 64
HO = WO = 30
NPIX = H * W            # 1024
XW = 1024               # working row width (pixels)
XIN = 1088              # input row width (1024 + 64 pad for row shifts)
ACW = 960               # accumulator width (30 rows x 32 cols)
POSW = 958              # last used pos col is 29*32+29 = 957
CW = 964                # chain compute width (cols actually consumed + pad)

SIG1 = 8.0              # stage-1 normalizer LSE sharpness (fits ACT Ln window)
SIG2 = 32.0             # stage-2 normalizer refinement sharpness
T = 112.0               # main LSE sharpness
G = 6.0                 # fixed global scale bound (|x| < 6 for N(0,1) data)
LG = float(np.log(G))
CSH = 216.0             # fp16 recentering shift for the log-domain combine
GCAP = 38.5             # cap on per-channel ln-rescale (ACT Ln window ~|44|)

F32 = mybir.dt.float32
F16 = mybir.dt.float16
BF16 = mybir.dt.bfloat16
_cache: dict = {}
last_results = None


def _ensure_axon_ntff_hook():
    """The trimmed agent image lacks antenv.axon_hooks; recreate it so
    run_bass_kernel_spmd(trace=True) can capture NTFF profiles. No-op on
    failure (tracing then just degrades)."""
    import sys
    import types

    try:
        import antenv.axon_hooks  # noqa: F401
        return
    except ImportError:
        pass
    try:
        mod = types.ModuleType("antenv.axon_hooks")
        holder = [None]
        mod.set_axon_ntff_profile_hook = lambda h: holder.__setitem__(0, h)
        mod.get_axon_ntff_profile_hook = lambda: holder[0]
        sys.modules["antenv.axon_hooks"] = mod
        from trn_agent_boot.trn_boot import _ntff_profile_via_ctypes

        so = "/opt/axon/libaxon_pjrt.so"
        if os.path.exists(so):
            holder[0] = _ntff_profile_via_ctypes(so)
    except Exception:
        pass


def _build_module():
    nc = _BaccOneActSet()
    Alu = mybir.AluOpType
    Act = mybir.ActivationFunctionType

    xT = nc.dram_tensor("xT", [C, XIN], F32, kind="ExternalInput")
    K3 = nc.dram_tensor("K3", [96, 384], BF16, kind="ExternalInput")
    MM = nc.dram_tensor("MM", [128, 128], F16, kind="ExternalInput")
    CM = nc.dram_tensor("CM", [128, 3 + 512], F32, kind="ExternalInput")
    Y = nc.dram_tensor("Y", [HO * WO, COUT], F32, kind="ExternalOutput")

    with tile.TileContext(nc) as tc, ExitStack() as ctx:
        const = ctx.enter_context(tc.tile_pool(name="const", bufs=1))
        sp = ctx.enter_context(tc.tile_pool(name="sp", bufs=1))
        losb = ctx.enter_context(tc.tile_pool(name="losb", bufs=3))
        accp = ctx.enter_context(tc.tile_pool(name="accp", bufs=1))
        vsb = ctx.enter_context(tc.tile_pool(name="vsb", bufs=1))
        msb = ctx.enter_context(tc.tile_pool(name="msb", bufs=1))
        ysb = ctx.enter_context(tc.tile_pool(name="ysb", bufs=2))
        psS = ctx.enter_context(tc.tile_pool(name="psS", bufs=2, space="PSUM"))
        psO = ctx.enter_context(tc.tile_pool(name="psO", bufs=2, space="PSUM"))

        # ---- constants / input staging ----
        xT3 = const.tile([96, XW], F32)
        for g, eng in ((0, nc.gpsimd), (1, nc.sync), (2, nc.scalar)):
            eng.dma_start(out=xT3[32 * g : 32 * g + 32, :],
                          in_=xT[:, 32 * g : 32 * g + XW])
        K3_sb = const.tile([96, 384], BF16)
        nc.sync.dma_start(out=K3_sb[:, :], in_=K3[:, :])
        MM_sb = const.tile([128, 128], F16)
        nc.gpsimd.dma_start(out=MM_sb[:, :], in_=MM[:, :])
        CM_sb = const.tile([128, 3 + 512], F32)
        nc.sync.dma_start(out=CM_sb[:, :], in_=CM[:, :])
        EXM_sb, SC_sb, AFL_sb = CM_sb[:, 0:1], CM_sb[:, 1:2], CM_sb[:, 2:3]
        BCW_sb = CM_sb[:, 3 : 3 + 512]
        ONES_sb = const.tile([96, 128], BF16)
        nc.gpsimd.memset(ONES_sb[:, :], 1.0)
        B8_sb = const.tile([96, 1], F32)
        nc.gpsimd.memset(B8_sb[:, :], -SIG1 * LG)
        B32_sb = const.tile([96, 1], F32)
        nc.gpsimd.memset(B32_sb[:, :], -SIG2 * LG)
        B96_sb = const.tile([96, 1], F32)
        nc.gpsimd.memset(B96_sb[:, :], -T * LG)

        m_tiles = []
        S = [dict(), dict()]
        for i, sgn in ((0, 1.0), (1, -1.0)):
            X3 = sp.tile([96, XW], F32, tag=f"X3{i}")
            nc.vector.tensor_scalar(out=X3[:, 0:CW], in0=xT3[:, 0:CW],
                                    scalar1=sgn, scalar2=0.1,
                                    op0=Alu.mult, op1=Alu.max)
            S[i]["X3"] = X3
        for i in (0, 1):
            lp3 = sp.tile([96, XW], F32, tag=f"lp3{i}")
            nc.scalar.activation(lp3[:, 0:CW], S[i]["X3"][:, 0:CW], Act.Ln)
            S[i]["lp3"] = lp3
        for i in (0, 1):
            E8 = sp.tile([96, XW], BF16, tag=f"E8{i}")
            nc.scalar.activation(E8[:, 0:CW], S[i]["lp3"][:, 0:CW], Act.Exp,
                                 bias=B8_sb[:, 0:1], scale=SIG1)
            S[i]["E8"] = E8
        for i in (0, 1):
            S8p = psS.tile([128, XW], F32, tag="Sp")
            nc.tensor.matmul(S8p[:, 0:512], lhsT=ONES_sb[:, :],
                             rhs=S[i]["E8"][:, 0:512], start=True, stop=True)
            nc.tensor.matmul(S8p[:, 512:CW], lhsT=ONES_sb[:, :],
                             rhs=S[i]["E8"][:, 512:CW], start=True, stop=True)
            S[i]["S8p"] = S8p
        for i in (0, 1):
            L8 = sp.tile([128, XW], F32, tag=f"L8{i}")
            nc.scalar.activation(L8[:, 0:CW], S[i]["S8p"][:, 0:CW], Act.Ln)
            S[i]["L8"] = L8
        for i in (0, 1):
            d8 = sp.tile([96, XW], F32, tag=f"d8{i}")
            nc.vector.scalar_tensor_tensor(out=d8[:, 0:CW], in0=S[i]["L8"][0:96, 0:CW],
                                           scalar=-1.0 / SIG1, in1=S[i]["lp3"][:, 0:CW],
                                           op0=Alu.mult, op1=Alu.add)
            S[i]["d8"] = d8
        for i in (0, 1):
            E32 = sp.tile([96, XW], BF16, tag=f"E32{i}")
            nc.scalar.activation(E32[:, 0:CW], S[i]["d8"][:, 0:CW], Act.Exp,
                                 bias=B32_sb[:, 0:1], scale=SIG2)
            S[i]["E32"] = E32
        for i in (0, 1):
            S32p = psS.tile([128, XW], F32, tag="Sp")
            nc.tensor.matmul(S32p[:, 0:512], lhsT=ONES_sb[:, :],
                             rhs=S[i]["E32"][:, 0:512], start=True, stop=True)
            nc.tensor.matmul(S32p[:, 512:CW], lhsT=ONES_sb[:, :],
                             rhs=S[i]["E32"][:, 512:CW], start=True, stop=True)
            S[i]["S32p"] = S32p
        for i in (0, 1):
            L32 = sp.tile([128, XW], F32, tag=f"L32{i}")
            nc.scalar.activation(L32[:, 0:CW], S[i]["S32p"][:, 0:CW], Act.Ln)
            S[i]["L32"] = L32
        for i in (0, 1):
            # d96 = lp - M3q + lG = d8 - L32/SIG2  (critical path, DVE)
            d96 = sp.tile([96, XW], F32, tag=f"d96{i}")
            nc.vector.scalar_tensor_tensor(out=d96[:, 0:CW], in0=S[i]["L32"][0:96, 0:CW],
                                           scalar=-1.0 / SIG2, in1=S[i]["d8"][:, 0:CW],
                                           op0=Alu.mult, op1=Alu.add)
            S[i]["d96"] = d96
        for i in (0, 1):
            E96 = sp.tile([96, XW], BF16, tag=f"E96{i}")
            nc.scalar.activation(E96[:, 0:CW], S[i]["d96"][:, 0:CW], Act.Exp,
                                 bias=B96_sb[:, 0:1], scale=T)
            S[i]["E96"] = E96
        for i in (0, 1):
            # T16 = (T/SIG1)*L8 + (T/SIG2)*L32 + CSH, off critical path (GpSimd)
            LS0 = sp.tile([128, XW], F32, tag=f"LS0{i}")
            nc.vector.scalar_tensor_tensor(out=LS0[:, 0:CW], in0=S[i]["L8"][:, 0:CW],
                                           scalar=SIG2 / SIG1, in1=S[i]["L32"][:, 0:CW],
                                           op0=Alu.mult, op1=Alu.add)
            T16e = sp.tile([128, XW], F16, tag=f"T16e{i}")
            nc.vector.tensor_scalar(out=T16e[:, 0:CW], in0=LS0[:, 0:CW],
                                    scalar1=T / SIG2, scalar2=CSH,
                                    op0=Alu.mult, op1=Alu.add)
            S[i]["T16e"] = T16e
            acc = accp.tile([128, ACW], F16, tag=f"acc{i}")
            S[i]["acc"] = acc
        for j in range(3):
            for i in (0, 1):
                Oj = psO.tile([128, XW], F32, tag="Oj")
                E96 = S[i]["E96"]
                nc.tensor.matmul(Oj[:, 0:512], lhsT=K3_sb[:, 128 * j : 128 * j + 128],
                                 rhs=E96[:, j : j + 512], start=True, stop=True)
                nc.tensor.matmul(Oj[:, 512:ACW], lhsT=K3_sb[:, 128 * j : 128 * j + 128],
                                 rhs=E96[:, j + 512 : j + ACW], start=True, stop=True)
                S[i]["Oj"] = Oj
            for i in (0, 1):
                LoS = losb.tile([128, ACW], F16, tag=f"LoS{i}")
                nc.scalar.activation(LoS[:, :], S[i]["Oj"][:, 0:ACW], Act.Ln,
                                     scale=SC_sb)
                S[i]["LoS"] = LoS
            for i in (0, 1):
                acc = S[i]["acc"]
                LoS = S[i]["LoS"]
                t16 = S[i]["T16e"]
                toff = j
                if j == 0:
                    nc.vector.tensor_tensor(
                        acc[:, 0:POSW], LoS[:, 0:POSW],
                        t16[:, toff : toff + POSW], Alu.add)
                else:
                    V = vsb.tile([128, POSW], F16, tag=f"V{i}")
                    nc.vector.tensor_tensor(
                        V[:, :], LoS[:, 0:POSW],
                        t16[:, toff : toff + POSW], Alu.add)
                    nc.vector.tensor_tensor(
                        acc[:, 0:POSW], V[:, :], acc[:, 0:POSW], Alu.max)
        for i in (0, 1):
            acc = S[i]["acc"]
            nc.vector.tensor_scalar(out=acc[:, 0:POSW], in0=acc[:, 0:POSW],
                                    scalar1=AFL_sb, scalar2=None,
                                    op0=Alu.max)
            m = msb.tile([128, HO * WO], F16, tag=f"m{i}")
            nc.scalar.activation(
                m.rearrange("q (a b) -> q a b", a=HO),
                acc.rearrange("q (a b) -> q a b", b=W)[:, :, :WO],
                Act.Exp, bias=EXM_sb, scale=1.0 / T)
            m_tiles.append(m)

        # combine + transpose into ONE psum bank, column-chunked:
        # ptall[p, 64*ci + u] = y[128*ci + p, u]
        m1, m2 = m_tiles
        ptall = psS.tile([128, XW], F32, tag="Sp")
        chunks = [(ci, min(128, HO * WO - 128 * ci)) for ci in range(8)]
        for ci, cw in chunks:
            nc.tensor.matmul(ptall[:cw, COUT * ci : COUT * ci + COUT],
                             lhsT=m1[:, 128 * ci : 128 * ci + cw],
                             rhs=MM_sb[:, 0:COUT], start=True, stop=False)
            nc.tensor.matmul(ptall[:cw, COUT * ci : COUT * ci + COUT],
                             lhsT=m2[:, 128 * ci : 128 * ci + cw],
                             rhs=MM_sb[:, COUT:128], start=False, stop=True)
        ytall = ysb.tile([128, 512], F32, tag="yt")
        nc.vector.tensor_tensor(ytall[:, :], ptall[:, 0:512], BCW_sb, Alu.add)
        for ci, cw in chunks:
            nc.sync.dma_start(out=Y[128 * ci : 128 * ci + cw, :],
                              in_=ytall[:cw, COUT * ci : COUT * ci + COUT])
    nc.finalize()
    return nc


def _host_prep(x, k1, k2, bias):
    x = np.ascontiguousarray(np.asarray(x, dtype=np.float32))
    k1 = np.asarray(k1, np.float32).reshape(3, 3, C, COUT)
    k2 = np.asarray(k2, np.float32).reshape(3, 3, C, COUT)
    Mk1 = k1.reshape(-1, COUT).max(axis=0)
    Mk2 = k2.reshape(-1, COUT).max(axis=0)
    K3 = np.zeros((96, 384), np.float32)  # cast to bf16 below
    for j in range(3):
        for g in range(3):
            K3[32 * g : 32 * g + 32, 128 * j : 128 * j + 64] = \
                np.exp(T * (k1[g, j] - Mk1))
            K3[32 * g : 32 * g + 32, 128 * j + 64 : 128 * j + 128] = \
                np.exp(T * (k2[g, j] - Mk2))
    I64 = np.eye(COUT, dtype=np.float16)
    MM = np.zeros((128, 128), np.float16)
    MM[0:64, 0:COUT] = I64
    MM[64:128, 0:COUT] = -I64
    MM[0:64, COUT:128] = -I64
    MM[64:128, COUT:128] = I64
    rng1 = Mk1 - k1.reshape(-1, COUT).min(axis=0)
    rng2 = Mk2 - k2.reshape(-1, COUT).min(axis=0)
    gc = np.minimum((16.2 + T * np.concatenate([rng1, rng2]) - 5.0) / 2.0, GCAP)
    Mk = np.concatenate([Mk1, Mk2])
    CM = np.zeros((128, 3 + 512), np.float32)
    CM[:, 0] = Mk + LG - (CSH + gc) / T
    CM[:, 1] = np.exp(gc)
    CM[:, 2] = T * (np.log(0.1) - LG) + CSH + gc
    CM[:, 3:] = np.tile(np.asarray(bias, np.float32).reshape(1, COUT), (1, 8))
    K3bf = (K3.view(np.uint32) >> 16).astype(np.uint16)  # truncate-to-bf16
    import ml_dtypes
    K3bf = K3.astype(ml_dtypes.bfloat16)
    shared = dict(K3=K3bf, MM=MM, CM=CM)
    in_maps = []
    for n in range(N_CORES):
        xT = np.zeros((C, XIN), np.float32)
        xT[:, :NPIX] = x[n].reshape(NPIX, C).T
        in_maps.append({"xT": xT, **shared})
    return in_maps


def kernel(x, k1, k2, bias):
    global last_results
    if "nc" not in _cache:
        _cache["nc"] = _build_module()
    nc = _cache["nc"]
    in_maps = _host_prep(x, k1, k2, bias)
    trace = bool(int(os.environ.get("KTRACE", "0")))
    if trace:
        _ensure_axon_ntff_hook()
    res = run_bass_kernel_spmd(
        nc, in_maps, core_ids=list(range(N_CORES)), trace=trace,
    )
    last_results = res
    y = np.stack([r["Y"].reshape(HO, WO, COUT) for r in res.results], axis=0)
    return y.astype(np.float32)


# revision 32
# speedup vs baseline: 12.4765x; 1.3002x over previous
"""Bipolar morphological conv2d kernel for Trainium2 (8 NeuronCores).

Math: reference computes, per output position and out-channel c,
    y = m(lp1,K1) - m(lp1,K2) - m(lp2,K1) + m(lp2,K2) + bias
with m(logp, k)[c] = exp(max_p(logp_p + k_pc)), lp1 = log(max(x, .1)),
lp2 = log(max(-x, .1)).

Device algorithm (data-parallel, one batch image per core): the 288-tap
max-plus reduction is evaluated as a tight LSE (p-norm, t=112) over each
3x1 column group of the 3x3 window (96 entries: 3 rows x 32 channels),
turning the heavy reduction into THREE K=96 TensorE matmuls, followed by
an EXACT max over the 3 column groups in log domain (DVE fp16).  A
per-rhs-column normalizer M3q (itself a sigma=32 LSE, computed with a
ones-matmul) keeps every exponential in fp32 range; its value cancels
exactly in the algebra, so only over/underflow matters, not its accuracy.

Layout: channel-major pixel rows.  xT3 [96, 1024] holds the 3 row-shifted
copies of xT [32, 1024] (partition 32g+ci = channel ci shifted g rows),
so a column-group matmul contracts all 96 entries in one instruction and
tap shifts become free-dim column offsets (multiples of 1).  Out-channels
of K1|K2 are stacked on the 128 output partitions, so one matmul chain
serves both kernels.  Final exp folds the per-channel max-k and the
30x30 window selection; a pair of +/-I matmuls transposes to
position-major and combines the 4 morphs with their signs.
"""

import os
from contextlib import ExitStack

import numpy as np

import concourse.bass as bass
import concourse.mybir as mybir
from concourse import bacc
import concourse.tile as tile
from concourse.bass_utils import run_bass_kernel_spmd
from concourse.hw_specs import get_activation_tables
import bass_rust as _bass_rust


class _BaccOneActSet(bacc.Bacc):
    """Bacc whose act-table pass may only pick natural_log_exp_and_others
    (contains Ln+Exp+Copy, the only functions used here), so the table is
    loaded once instead of thrashing on every Ln<->Exp transition."""

    def insert_act_table_loads(self):
        has_activation = any(
            isinstance(i, mybir.InstActivation)
            for b in self.main_func.blocks
            for i in b.instructions
        )
        if not has_activation:
            return
        tables = [
            (n, (fns if n == "natural_log_exp_and_others" else set()))
            for n, fns in get_activation_tables(self.m.arch).items()
        ]
        _bass_rust.insert_act_table_loads(self, tables)

N_CORES = 8
H = W = C = 32
COUT = 64
HO = WO = 30
NPIX = H * W            # 1024
XW = 1024               # working row width (pixels)
XIN = 1088              # input row width (1024 + 64 pad for row shifts)
ACW = 960               # accumulator width (30 rows x 32 cols)
POSW = 958              # last used pos col is 29*32+29 = 957
CW = 964                # chain compute width (cols actually consumed + pad)

SIG1 = 8.0              # stage-1 normalizer LSE sharpness (fits ACT Ln window)
SIG2 = 32.0             # stage-2 normalizer refinement sharpness
T = 112.0               # main LSE sharpness
G = 6.0                 # fixed global scale bound (|x| < 6 for N(0,1) data)
LG = float(np.log(G))
CSH = 216.0             # fp16 recentering shift for the log-domain combine
GCAP = 38.5             # cap on per-channel ln-rescale (ACT Ln window ~|44|)

F32 = mybir.dt.float32
F16 = mybir.dt.float16
BF16 = mybir.dt.bfloat16
_cache: dict = {}
last_results = None


def _ensure_axon_ntff_hook():
    """The trimmed agent image lacks antenv.axon_hooks; recreate it so
    run_bass_kernel_spmd(trace=True) can capture NTFF profiles. No-op on
    failure (tracing then just degrades)."""
    import sys
    import types

    try:
        import antenv.axon_hooks  # noqa: F401
        return
    except ImportError:
        pass
    try:
        mod = types.ModuleType("antenv.axon_hooks")
        holder = [None]
        mod.set_axon_ntff_profile_hook = lambda h: holder.__setitem__(0, h)
        mod.get_axon_ntff_profile_hook = lambda: holder[0]
        sys.modules["antenv.axon_hooks"] = mod
        from trn_agent_boot.trn_boot import _ntff_profile_via_ctypes

        so = "/opt/axon/libaxon_pjrt.so"
        if os.path.exists(so):
            holder[0] = _ntff_profile_via_ctypes(so)
    except Exception:
        pass


def _build_module():
    nc = _BaccOneActSet()
    Alu = mybir.AluOpType
    Act = mybir.ActivationFunctionType

    xT = nc.dram_tensor("xT", [C, XIN], F32, kind="ExternalInput")
    K3 = nc.dram_tensor("K3", [96, 384], BF16, kind="ExternalInput")
    MM = nc.dram_tensor("MM", [128, 128], F16, kind="ExternalInput")
    CM = nc.dram_tensor("CM", [128, 3 + 512], F32, kind="ExternalInput")
    Y = nc.dram_tensor("Y", [128, 512], F32, kind="ExternalOutput")

    with tile.TileContext(nc) as tc, ExitStack() as ctx:
        const = ctx.enter_context(tc.tile_pool(name="const", bufs=1))
        sp = ctx.enter_context(tc.tile_pool(name="sp", bufs=1))
        losb = ctx.enter_context(tc.tile_pool(name="losb", bufs=3))
        accp = ctx.enter_context(tc.tile_pool(name="accp", bufs=1))
        vsb = ctx.enter_context(tc.tile_pool(name="vsb", bufs=1))
        msb = ctx.enter_context(tc.tile_pool(name="msb", bufs=1))
        ysb = ctx.enter_context(tc.tile_pool(name="ysb", bufs=2))
        psS = ctx.enter_context(tc.tile_pool(name="psS", bufs=2, space="PSUM"))
        psO = ctx.enter_context(tc.tile_pool(name="psO", bufs=2, space="PSUM"))

        # ---- constants / input staging ----
        xT3 = const.tile([96, XW], F32)
        for g, eng in ((0, nc.gpsimd), (1, nc.sync), (2, nc.scalar)):
            eng.dma_start(out=xT3[32 * g : 32 * g + 32, :],
                          in_=xT[:, 32 * g : 32 * g + XW])
        K3_sb = const.tile([96, 384], BF16)
        nc.sync.dma_start(out=K3_sb[:, :], in_=K3[:, :])
        MM_sb = const.tile([128, 128], F16)
        nc.gpsimd.dma_start(out=MM_sb[:, :], in_=MM[:, :])
        CM_sb = const.tile([128, 3 + 512], F32)
        nc.sync.dma_start(out=CM_sb[:, :], in_=CM[:, :])
        EXM_sb, SC_sb, AFL_sb = CM_sb[:, 0:1], CM_sb[:, 1:2], CM_sb[:, 2:3]
        BCW_sb = CM_sb[:, 3 : 3 + 512]
        ONES_sb = const.tile([96, 128], BF16)
        nc.gpsimd.memset(ONES_sb[:, :], 1.0)
        B8_sb = const.tile([96, 1], F32)
        nc.gpsimd.memset(B8_sb[:, :], -SIG1 * LG)
        B32_sb = const.tile([96, 1], F32)
        nc.gpsimd.memset(B32_sb[:, :], -SIG2 * LG)
        B96_sb = const.tile([96, 1], F32)
        nc.gpsimd.memset(B96_sb[:, :], -T * LG)

        m_tiles = []
        S = [dict(), dict()]
        for i, sgn in ((0, 1.0), (1, -1.0)):
            X3 = sp.tile([96, XW], F32, tag=f"X3{i}")
            nc.vector.tensor_scalar(out=X3[:, 0:CW], in0=xT3[:, 0:CW],
                                    scalar1=sgn, scalar2=0.1,
                                    op0=Alu.mult, op1=Alu.max)
            S[i]["X3"] = X3
        for i in (0, 1):
            lp3 = sp.tile([96, XW], F32, tag=f"lp3{i}")
            nc.scalar.activation(lp3[:, 0:CW], S[i]["X3"][:, 0:CW], Act.Ln)
            S[i]["lp3"] = lp3
        for i in (0, 1):
            E8 = sp.tile([96, XW], BF16, tag=f"E8{i}")
            nc.scalar.activation(E8[:, 0:CW], S[i]["lp3"][:, 0:CW], Act.Exp,
                                 bias=B8_sb[:, 0:1], scale=SIG1)
            S[i]["E8"] = E8
        for i in (0, 1):
            S8p = psS.tile([128, XW], F32, tag="Sp")
            nc.tensor.matmul(S8p[:, 0:512], lhsT=ONES_sb[:, :],
                             rhs=S[i]["E8"][:, 0:512], start=True, stop=True)
            nc.tensor.matmul(S8p[:, 512:CW], lhsT=ONES_sb[:, :],
                             rhs=S[i]["E8"][:, 512:CW], start=True, stop=True)
            S[i]["S8p"] = S8p
        for i in (0, 1):
            L8 = sp.tile([128, XW], F32, tag=f"L8{i}")
            nc.scalar.activation(L8[:, 0:CW], S[i]["S8p"][:, 0:CW], Act.Ln)
            S[i]["L8"] = L8
        for i in (0, 1):
            d8 = sp.tile([96, XW], F32, tag=f"d8{i}")
            nc.vector.scalar_tensor_tensor(out=d8[:, 0:CW], in0=S[i]["L8"][0:96, 0:CW],
                                           scalar=-1.0 / SIG1, in1=S[i]["lp3"][:, 0:CW],
                                           op0=Alu.mult, op1=Alu.add)
            S[i]["d8"] = d8
        for i in (0, 1):
            E32 = sp.tile([96, XW], BF16, tag=f"E32{i}")
            nc.scalar.activation(E32[:, 0:CW], S[i]["d8"][:, 0:CW], Act.Exp,
                                 bias=B32_sb[:, 0:1], scale=SIG2)
            S[i]["E32"] = E32
        for i in (0, 1):
            S32p = psS.tile([128, XW], F32, tag="Sp")
            nc.tensor.matmul(S32p[:, 0:512], lhsT=ONES_sb[:, :],
                             rhs=S[i]["E32"][:, 0:512], start=True, stop=True)
            nc.tensor.matmul(S32p[:, 512:CW], lhsT=ONES_sb[:, :],
                             rhs=S[i]["E32"][:, 512:CW], start=True, stop=True)
            S[i]["S32p"] = S32p
        for i in (0, 1):
            L32 = sp.tile([128, XW], F32, tag=f"L32{i}")
            nc.scalar.activation(L32[:, 0:CW], S[i]["S32p"][:, 0:CW], Act.Ln)
            S[i]["L32"] = L32
        for i in (0, 1):
            # d96 = lp - M3q + lG = d8 - L32/SIG2  (critical path, DVE)
            d96 = sp.tile([96, XW], F32, tag=f"d96{i}")
            nc.vector.scalar_tensor_tensor(out=d96[:, 0:CW], in0=S[i]["L32"][0:96, 0:CW],
                                           scalar=-1.0 / SIG2, in1=S[i]["d8"][:, 0:CW],
                                           op0=Alu.mult, op1=Alu.add)
            S[i]["d96"] = d96
        for i in (0, 1):
            E96 = sp.tile([96, XW], BF16, tag=f"E96{i}")
            nc.scalar.activation(E96[:, 0:CW], S[i]["d96"][:, 0:CW], Act.Exp,
                                 bias=B96_sb[:, 0:1], scale=T)
            S[i]["E96"] = E96
        for i in (0, 1):
            # T16 = (T/SIG1)*L8 + (T/SIG2)*L32 + CSH, off critical path (GpSimd)
            LS0 = sp.tile([128, XW], F32, tag=f"LS0{i}")
            nc.vector.scalar_tensor_tensor(out=LS0[:, 0:CW], in0=S[i]["L8"][:, 0:CW],
                                           scalar=SIG2 / SIG1, in1=S[i]["L32"][:, 0:CW],
                                           op0=Alu.mult, op1=Alu.add)
            T16e = sp.tile([128, XW], F16, tag=f"T16e{i}")
            nc.vector.tensor_scalar(out=T16e[:, 0:CW], in0=LS0[:, 0:CW],
                                    scalar1=T / SIG2, scalar2=CSH,
                                    op0=Alu.mult, op1=Alu.add)
            S[i]["T16e"] = T16e
            acc = accp.tile([128, ACW], F16, tag=f"acc{i}")
            S[i]["acc"] = acc
        for j in range(3):
            for i in (0, 1):
                Oj = psO.tile([128, XW], F32, tag="Oj")
                E96 = S[i]["E96"]
                nc.tensor.matmul(Oj[:, 0:512], lhsT=K3_sb[:, 128 * j : 128 * j + 128],
                                 rhs=E96[:, j : j + 512], start=True, stop=True)
                nc.tensor.matmul(Oj[:, 512:ACW], lhsT=K3_sb[:, 128 * j : 128 * j + 128],
                                 rhs=E96[:, j + 512 : j + ACW], start=True, stop=True)
                S[i]["Oj"] = Oj
            for i in (0, 1):
                LoS = losb.tile([128, ACW], F16, tag=f"LoS{i}")
                nc.scalar.activation(LoS[:, :], S[i]["Oj"][:, 0:ACW], Act.Ln,
                                     scale=SC_sb)
                S[i]["LoS"] = LoS
            for i in (0, 1):
                acc = S[i]["acc"]
                LoS = S[i]["LoS"]
                t16 = S[i]["T16e"]
                toff = j
                if j == 0:
                    nc.vector.tensor_tensor(
                        acc[:, 0:POSW], LoS[:, 0:POSW],
                        t16[:, toff : toff + POSW], Alu.add)
                else:
                    V = vsb.tile([128, POSW], F16, tag=f"V{i}")
                    nc.vector.tensor_tensor(
                        V[:, :], LoS[:, 0:POSW],
                        t16[:, toff : toff + POSW], Alu.add)
                    nc.vector.tensor_tensor(
                        acc[:, 0:POSW], V[:, :], acc[:, 0:POSW], Alu.max)
        for i in (0, 1):
            acc = S[i]["acc"]
            nc.vector.tensor_scalar(out=acc[:, 0:POSW], in0=acc[:, 0:POSW],
                                    scalar1=AFL_sb, scalar2=None,
                                    op0=Alu.max)
            m = msb.tile([128, HO * WO], F16, tag=f"m{i}")
            nc.scalar.activation(
                m.rearrange("q (a b) -> q a b", a=HO),
                acc.rearrange("q (a b) -> q a b", b=W)[:, :, :WO],
                Act.Exp, bias=EXM_sb, scale=1.0 / T)
            m_tiles.append(m)

        # combine + transpose into ONE psum bank, column-chunked:
        # ptall[p, 64*ci + u] = y[128*ci + p, u]
        m1, m2 = m_tiles
        ptall = psS.tile([128, XW], F32, tag="Sp")
        chunks = [(ci, min(128, HO * WO - 128 * ci)) for ci in range(8)]
        for ci, cw in chunks:
            nc.tensor.matmul(ptall[:cw, COUT * ci : COUT * ci + COUT],
                             lhsT=m1[:, 128 * ci : 128 * ci + cw],
                             rhs=MM_sb[:, 0:COUT], start=True, stop=False)
            nc.tensor.matmul(ptall[:cw, COUT * ci : COUT * ci + COUT],
                             lhsT=m2[:, 128 * ci : 128 * ci + cw],
                             rhs=MM_sb[:, COUT:128], start=False, stop=True)
        ytall = ysb.tile([128, 512], F32, tag="yt")
        nc.vector.tensor_tensor(ytall[:, :], ptall[:, 0:512], BCW_sb, Alu.add)
        nc.sync.dma_start(out=Y[:, :], in_=ytall[:, :])
    nc.finalize()
    return nc


def _host_prep(x, k1, k2, bias):
    x = np.ascontiguousarray(np.asarray(x, dtype=np.float32))
    k1 = np.asarray(k1, np.float32).reshape(3, 3, C, COUT)
    k2 = np.asarray(k2, np.float32).reshape(3, 3, C, COUT)
    Mk1 = k1.reshape(-1, COUT).max(axis=0)
    Mk2 = k2.reshape(-1, COUT).max(axis=0)
    K3 = np.zeros((96, 384), np.float32)  # cast to bf16 below
    for j in range(3):
        for g in range(3):
            K3[32 * g : 32 * g + 32, 128 * j : 128 * j + 64] = \
                np.exp(T * (k1[g, j] - Mk1))
            K3[32 * g : 32 * g + 32, 128 * j + 64 : 128 * j + 128] = \
                np.exp(T * (k2[g, j] - Mk2))
    I64 = np.eye(COUT, dtype=np.float16)
    MM = np.zeros((128, 128), np.float16)
    MM[0:64, 0:COUT] = I64
    MM[64:128, 0:COUT] = -I64
    MM[0:64, COUT:128] = -I64
    MM[64:128, COUT:128] = I64
    rng1 = Mk1 - k1.reshape(-1, COUT).min(axis=0)
    rng2 = Mk2 - k2.reshape(-1, COUT).min(axis=0)
    gc = np.minimum((16.2 + T * np.concatenate([rng1, rng2]) - 5.0) / 2.0, GCAP)
    Mk = np.concatenate([Mk1, Mk2])
    CM = np.zeros((128, 3 + 512), np.float32)
    CM[:, 0] = Mk + LG - (CSH + gc) / T
    CM[:, 1] = np.exp(gc)
    CM[:, 2] = T * (np.log(0.1) - LG) + CSH + gc
    CM[:, 3:] = np.tile(np.asarray(bias, np.float32).reshape(1, COUT), (1, 8))
    import ml_dtypes
    K3bf = K3.astype(ml_dtypes.bfloat16)
    shared = dict(K3=K3bf, MM=MM, CM=CM)
    in_maps = []
    for n in range(N_CORES):
        xT = np.zeros((C, XIN), np.float32)
        xT[:, :NPIX] = x[n].reshape(NPIX, C).T
        in_maps.append({"xT": xT, **shared})
    return in_maps


def kernel(x, k1, k2, bias):
    global last_results
    if "nc" not in _cache:
        _cache["nc"] = _build_module()
    nc = _cache["nc"]
    in_maps = _host_prep(x, k1, k2, bias)
    trace = bool(int(os.environ.get("KTRACE", "0")))
    if trace:
        _ensure_axon_ntff_hook()
    res = run_bass_kernel_spmd(
        nc, in_maps, core_ids=list(range(N_CORES)), trace=trace,
    )
    last_results = res
    # Y[p, 64*ci + u] = y[128*ci + p, u]
    ys = []
    for r in res.results:
        yd = r["Y"].reshape(128, 8, COUT).transpose(1, 0, 2).reshape(1024, COUT)
        ys.append(yd[: HO * WO].reshape(HO, WO, COUT))
    return np.stack(ys, axis=0).astype(np.float32)


# revision 33
# speedup vs baseline: 12.5553x; 1.0063x over previous
"""Bipolar morphological conv2d kernel for Trainium2 (8 NeuronCores).

Math: reference computes, per output position and out-channel c,
    y = m(lp1,K1) - m(lp1,K2) - m(lp2,K1) + m(lp2,K2) + bias
with m(logp, k)[c] = exp(max_p(logp_p + k_pc)), lp1 = log(max(x, .1)),
lp2 = log(max(-x, .1)).

Device algorithm (data-parallel, one batch image per core): the 288-tap
max-plus reduction is evaluated as a tight LSE (p-norm, t=112) over each
3x1 column group of the 3x3 window (96 entries: 3 rows x 32 channels),
turning the heavy reduction into THREE K=96 TensorE matmuls, followed by
an EXACT max over the 3 column groups in log domain (DVE fp16).  A
per-rhs-column normalizer M3q (itself a sigma=32 LSE, computed with a
ones-matmul) keeps every exponential in fp32 range; its value cancels
exactly in the algebra, so only over/underflow matters, not its accuracy.

Layout: channel-major pixel rows.  xT3 [96, 1024] holds the 3 row-shifted
copies of xT [32, 1024] (partition 32g+ci = channel ci shifted g rows),
so a column-group matmul contracts all 96 entries in one instruction and
tap shifts become free-dim column offsets (multiples of 1).  Out-channels
of K1|K2 are stacked on the 128 output partitions, so one matmul chain
serves both kernels.  Final exp folds the per-channel max-k and the
30x30 window selection; a pair of +/-I matmuls transposes to
position-major and combines the 4 morphs with their signs.
"""

import os
from contextlib import ExitStack

import numpy as np

import concourse.bass as bass
import concourse.mybir as mybir
from concourse import bacc
import concourse.tile as tile
from concourse.bass_utils import run_bass_kernel_spmd
from concourse.hw_specs import get_activation_tables
import bass_rust as _bass_rust


class _BaccOneActSet(bacc.Bacc):
    """Bacc whose act-table pass may only pick natural_log_exp_and_others
    (contains Ln+Exp+Copy, the only functions used here), so the table is
    loaded once instead of thrashing on every Ln<->Exp transition."""

    def insert_act_table_loads(self):
        has_activation = any(
            isinstance(i, mybir.InstActivation)
            for b in self.main_func.blocks
            for i in b.instructions
        )
        if not has_activation:
            return
        tables = [
            (n, (fns if n == "natural_log_exp_and_others" else set()))
            for n, fns in get_activation_tables(self.m.arch).items()
        ]
        _bass_rust.insert_act_table_loads(self, tables)

N_CORES = 8
H = W = C = 32
COUT = 64
HO = WO = 30
NPIX = H * W            # 1024
XW = 1024               # working row width (pixels)
XIN = 1088              # input row width (1024 + 64 pad for row shifts)
ACW = 960               # accumulator width (30 rows x 32 cols)
POSW = 958              # last used pos col is 29*32+29 = 957
CW = 964                # chain compute width (cols actually consumed + pad)

SIG1 = 8.0              # stage-1 normalizer LSE sharpness (fits ACT Ln window)
SIG2 = 32.0             # stage-2 normalizer refinement sharpness
T = 112.0               # main LSE sharpness
G = 6.0                 # fixed global scale bound (|x| < 6 for N(0,1) data)
LG = float(np.log(G))
CSH = 216.0             # fp16 recentering shift for the log-domain combine
GCAP = 38.5             # cap on per-channel ln-rescale (ACT Ln window ~|44|)

F32 = mybir.dt.float32
F16 = mybir.dt.float16
BF16 = mybir.dt.bfloat16
_cache: dict = {}
last_results = None


def _ensure_axon_ntff_hook():
    """The trimmed agent image lacks antenv.axon_hooks; recreate it so
    run_bass_kernel_spmd(trace=True) can capture NTFF profiles. No-op on
    failure (tracing then just degrades)."""
    import sys
    import types

    try:
        import antenv.axon_hooks  # noqa: F401
        return
    except ImportError:
        pass
    try:
        mod = types.ModuleType("antenv.axon_hooks")
        holder = [None]
        mod.set_axon_ntff_profile_hook = lambda h: holder.__setitem__(0, h)
        mod.get_axon_ntff_profile_hook = lambda: holder[0]
        sys.modules["antenv.axon_hooks"] = mod
        from trn_agent_boot.trn_boot import _ntff_profile_via_ctypes

        so = "/opt/axon/libaxon_pjrt.so"
        if os.path.exists(so):
            holder[0] = _ntff_profile_via_ctypes(so)
    except Exception:
        pass


def _build_module():
    nc = _BaccOneActSet()
    Alu = mybir.AluOpType
    Act = mybir.ActivationFunctionType

    xT = nc.dram_tensor("xT", [C, XIN], F32, kind="ExternalInput")
    K3 = nc.dram_tensor("K3", [96, 384], BF16, kind="ExternalInput")
    MM = nc.dram_tensor("MM", [128, 128], F16, kind="ExternalInput")
    CM = nc.dram_tensor("CM", [128, 3 + 512], F32, kind="ExternalInput")
    Y = nc.dram_tensor("Y", [128, 512], F32, kind="ExternalOutput")

    with tile.TileContext(nc) as tc, ExitStack() as ctx:
        const = ctx.enter_context(tc.tile_pool(name="const", bufs=1))
        sp = ctx.enter_context(tc.tile_pool(name="sp", bufs=1))
        losb = ctx.enter_context(tc.tile_pool(name="losb", bufs=3))
        accp = ctx.enter_context(tc.tile_pool(name="accp", bufs=1))
        vsb = ctx.enter_context(tc.tile_pool(name="vsb", bufs=1))
        msb = ctx.enter_context(tc.tile_pool(name="msb", bufs=1))
        ysb = ctx.enter_context(tc.tile_pool(name="ysb", bufs=2))
        psS = ctx.enter_context(tc.tile_pool(name="psS", bufs=2, space="PSUM"))
        psO = ctx.enter_context(tc.tile_pool(name="psO", bufs=2, space="PSUM"))

        # ---- constants / input staging ----
        xT3 = const.tile([96, XW], F32)
        for g, eng in ((0, nc.gpsimd), (1, nc.sync), (2, nc.sync)):
            eng.dma_start(out=xT3[32 * g : 32 * g + 32, :],
                          in_=xT[:, 32 * g : 32 * g + XW])
        K3_sb = const.tile([96, 384], BF16)
        nc.sync.dma_start(out=K3_sb[:, :], in_=K3[:, :])
        MM_sb = const.tile([128, 128], F16)
        nc.gpsimd.dma_start(out=MM_sb[:, :], in_=MM[:, :])
        CM_sb = const.tile([128, 3 + 512], F32)
        nc.sync.dma_start(out=CM_sb[:, :], in_=CM[:, :])
        EXM_sb, SC_sb, AFL_sb = CM_sb[:, 0:1], CM_sb[:, 1:2], CM_sb[:, 2:3]
        BCW_sb = CM_sb[:, 3 : 3 + 512]
        ONES_sb = const.tile([96, 128], BF16)
        nc.vector.memset(ONES_sb[:, :], 1.0)
        B8_sb = const.tile([96, 1], F32)
        nc.vector.memset(B8_sb[:, :], -SIG1 * LG)
        B32_sb = const.tile([96, 1], F32)
        nc.vector.memset(B32_sb[:, :], -SIG2 * LG)
        B96_sb = const.tile([96, 1], F32)
        nc.vector.memset(B96_sb[:, :], -T * LG)

        # PE warm-up: junk matmuls bridge the startup window so HAM
        # reaches K=8/8 before the first real matmul (and stays there).
        Wm = const.tile([128, 512], BF16)
        nc.vector.memset(Wm[:, :], 1.0)
        Wp = psS.tile([128, XW], F32, tag="Sp")
        for w in range(24):
            nc.tensor.matmul(Wp[:, 0:512], lhsT=Wm[:, 0:128], rhs=Wm[:, :],
                             start=True, stop=True)

        m_tiles = []
        S = [dict(), dict()]
        for i, sgn in ((0, 1.0), (1, -1.0)):
            X3 = sp.tile([96, XW], F32, tag=f"X3{i}")
            nc.vector.tensor_scalar(out=X3[:, 0:CW], in0=xT3[:, 0:CW],
                                    scalar1=sgn, scalar2=0.1,
                                    op0=Alu.mult, op1=Alu.max)
            S[i]["X3"] = X3
        for i in (0, 1):
            lp3 = sp.tile([96, XW], F32, tag=f"lp3{i}")
            nc.scalar.activation(lp3[:, 0:CW], S[i]["X3"][:, 0:CW], Act.Ln)
            S[i]["lp3"] = lp3
        for i in (0, 1):
            E8 = sp.tile([96, XW], BF16, tag=f"E8{i}")
            nc.scalar.activation(E8[:, 0:CW], S[i]["lp3"][:, 0:CW], Act.Exp,
                                 bias=B8_sb[:, 0:1], scale=SIG1)
            S[i]["E8"] = E8
        for i in (0, 1):
            S8p = psS.tile([128, XW], F32, tag="Sp")
            nc.tensor.matmul(S8p[:, 0:512], lhsT=ONES_sb[:, :],
                             rhs=S[i]["E8"][:, 0:512], start=True, stop=True)
            nc.tensor.matmul(S8p[:, 512:CW], lhsT=ONES_sb[:, :],
                             rhs=S[i]["E8"][:, 512:CW], start=True, stop=True)
            S[i]["S8p"] = S8p
        for i in (0, 1):
            L8 = sp.tile([128, XW], F32, tag=f"L8{i}")
            nc.scalar.activation(L8[:, 0:CW], S[i]["S8p"][:, 0:CW], Act.Ln)
            S[i]["L8"] = L8
        for i in (0, 1):
            d8 = sp.tile([96, XW], F32, tag=f"d8{i}")
            nc.vector.scalar_tensor_tensor(out=d8[:, 0:CW], in0=S[i]["L8"][0:96, 0:CW],
                                           scalar=-1.0 / SIG1, in1=S[i]["lp3"][:, 0:CW],
                                           op0=Alu.mult, op1=Alu.add)
            S[i]["d8"] = d8
        for i in (0, 1):
            E32 = sp.tile([96, XW], BF16, tag=f"E32{i}")
            nc.scalar.activation(E32[:, 0:CW], S[i]["d8"][:, 0:CW], Act.Exp,
                                 bias=B32_sb[:, 0:1], scale=SIG2)
            S[i]["E32"] = E32
        for i in (0, 1):
            S32p = psS.tile([128, XW], F32, tag="Sp")
            nc.tensor.matmul(S32p[:, 0:512], lhsT=ONES_sb[:, :],
                             rhs=S[i]["E32"][:, 0:512], start=True, stop=True)
            nc.tensor.matmul(S32p[:, 512:CW], lhsT=ONES_sb[:, :],
                             rhs=S[i]["E32"][:, 512:CW], start=True, stop=True)
            S[i]["S32p"] = S32p
        for i in (0, 1):
            L32 = sp.tile([128, XW], F32, tag=f"L32{i}")
            nc.scalar.activation(L32[:, 0:CW], S[i]["S32p"][:, 0:CW], Act.Ln)
            S[i]["L32"] = L32
        for i in (0, 1):
            # d96 = lp - M3q + lG = d8 - L32/SIG2  (critical path, DVE)
            d96 = sp.tile([96, XW], F32, tag=f"d96{i}")
            nc.vector.scalar_tensor_tensor(out=d96[:, 0:CW], in0=S[i]["L32"][0:96, 0:CW],
                                           scalar=-1.0 / SIG2, in1=S[i]["d8"][:, 0:CW],
                                           op0=Alu.mult, op1=Alu.add)
            S[i]["d96"] = d96
        for i in (0, 1):
            E96 = sp.tile([96, XW], BF16, tag=f"E96{i}")
            nc.scalar.activation(E96[:, 0:CW], S[i]["d96"][:, 0:CW], Act.Exp,
                                 bias=B96_sb[:, 0:1], scale=T)
            S[i]["E96"] = E96
        for i in (0, 1):
            # T16 = (T/SIG1)*L8 + (T/SIG2)*L32 + CSH, off critical path (GpSimd)
            LS0 = sp.tile([128, XW], F32, tag=f"LS0{i}")
            nc.vector.scalar_tensor_tensor(out=LS0[:, 0:CW], in0=S[i]["L8"][:, 0:CW],
                                           scalar=SIG2 / SIG1, in1=S[i]["L32"][:, 0:CW],
                                           op0=Alu.mult, op1=Alu.add)
            T16e = sp.tile([128, XW], F16, tag=f"T16e{i}")
            nc.vector.tensor_scalar(out=T16e[:, 0:CW], in0=LS0[:, 0:CW],
                                    scalar1=T / SIG2, scalar2=CSH,
                                    op0=Alu.mult, op1=Alu.add)
            S[i]["T16e"] = T16e
            acc = accp.tile([128, ACW], F16, tag=f"acc{i}")
            S[i]["acc"] = acc
        for j in range(3):
            for i in (0, 1):
                Oj = psO.tile([128, XW], F32, tag="Oj")
                E96 = S[i]["E96"]
                nc.tensor.matmul(Oj[:, 0:512], lhsT=K3_sb[:, 128 * j : 128 * j + 128],
                                 rhs=E96[:, j : j + 512], start=True, stop=True)
                nc.tensor.matmul(Oj[:, 512:ACW], lhsT=K3_sb[:, 128 * j : 128 * j + 128],
                                 rhs=E96[:, j + 512 : j + ACW], start=True, stop=True)
                S[i]["Oj"] = Oj
            for i in (0, 1):
                LoS = losb.tile([128, ACW], F16, tag=f"LoS{i}")
                nc.scalar.activation(LoS[:, :], S[i]["Oj"][:, 0:ACW], Act.Ln,
                                     scale=SC_sb)
                S[i]["LoS"] = LoS
            for i in (0, 1):
                acc = S[i]["acc"]
                LoS = S[i]["LoS"]
                t16 = S[i]["T16e"]
                toff = j
                if j == 0:
                    nc.vector.tensor_tensor(
                        acc[:, 0:POSW], LoS[:, 0:POSW],
                        t16[:, toff : toff + POSW], Alu.add)
                else:
                    V = vsb.tile([128, POSW], F16, tag=f"V{i}")
                    nc.vector.tensor_tensor(
                        V[:, :], LoS[:, 0:POSW],
                        t16[:, toff : toff + POSW], Alu.add)
                    nc.vector.tensor_tensor(
                        acc[:, 0:POSW], V[:, :], acc[:, 0:POSW], Alu.max)
        for i in (0, 1):
            acc = S[i]["acc"]
            nc.vector.tensor_scalar(out=acc[:, 0:POSW], in0=acc[:, 0:POSW],
                                    scalar1=AFL_sb, scalar2=None,
                                    op0=Alu.max)
            m = msb.tile([128, HO * WO], F16, tag=f"m{i}")
            nc.scalar.activation(
                m.rearrange("q (a b) -> q a b", a=HO),
                acc.rearrange("q (a b) -> q a b", b=W)[:, :, :WO],
                Act.Exp, bias=EXM_sb, scale=1.0 / T)
            m_tiles.append(m)

        # combine + transpose into ONE psum bank, column-chunked:
        # ptall[p, 64*ci + u] = y[128*ci + p, u]
        m1, m2 = m_tiles
        ptall = psS.tile([128, XW], F32, tag="Sp")
        chunks = [(ci, min(128, HO * WO - 128 * ci)) for ci in range(8)]
        for ci, cw in chunks:
            nc.tensor.matmul(ptall[:cw, COUT * ci : COUT * ci + COUT],
                             lhsT=m1[:, 128 * ci : 128 * ci + cw],
                             rhs=MM_sb[:, 0:COUT], start=True, stop=False)
            nc.tensor.matmul(ptall[:cw, COUT * ci : COUT * ci + COUT],
                             lhsT=m2[:, 128 * ci : 128 * ci + cw],
                             rhs=MM_sb[:, COUT:128], start=False, stop=True)
        ytall = ysb.tile([128, 512], F32, tag="yt")
        nc.vector.tensor_tensor(ytall[:, :], ptall[:, 0:512], BCW_sb, Alu.add)
        nc.sync.dma_start(out=Y[:, :], in_=ytall[:, :])
    nc.finalize()
    return nc


def _host_prep(x, k1, k2, bias):
    x = np.ascontiguousarray(np.asarray(x, dtype=np.float32))
    k1 = np.asarray(k1, np.float32).reshape(3, 3, C, COUT)
    k2 = np.asarray(k2, np.float32).reshape(3, 3, C, COUT)
    Mk1 = k1.reshape(-1, COUT).max(axis=0)
    Mk2 = k2.reshape(-1, COUT).max(axis=0)
    K3 = np.zeros((96, 384), np.float32)  # cast to bf16 below
    for j in range(3):
        for g in range(3):
            K3[32 * g : 32 * g + 32, 128 * j : 128 * j + 64] = \
                np.exp(T * (k1[g, j] - Mk1))
            K3[32 * g : 32 * g + 32, 128 * j + 64 : 128 * j + 128] = \
                np.exp(T * (k2[g, j] - Mk2))
    I64 = np.eye(COUT, dtype=np.float16)
    MM = np.zeros((128, 128), np.float16)
    MM[0:64, 0:COUT] = I64
    MM[64:128, 0:COUT] = -I64
    MM[0:64, COUT:128] = -I64
    MM[64:128, COUT:128] = I64
    rng1 = Mk1 - k1.reshape(-1, COUT).min(axis=0)
    rng2 = Mk2 - k2.reshape(-1, COUT).min(axis=0)
    gc = np.minimum((16.2 + T * np.concatenate([rng1, rng2]) - 5.0) / 2.0, GCAP)
    Mk = np.concatenate([Mk1, Mk2])
    CM = np.zeros((128, 3 + 512), np.float32)
    CM[:, 0] = Mk + LG - (CSH + gc) / T
    CM[:, 1] = np.exp(gc)
    CM[:, 2] = T * (np.log(0.1) - LG) + CSH + gc
    CM[:, 3:] = np.tile(np.asarray(bias, np.float32).reshape(1, COUT), (1, 8))
    import ml_dtypes
    K3bf = K3.astype(ml_dtypes.bfloat16)
    shared = dict(K3=K3bf, MM=MM, CM=CM)
    in_maps = []
    for n in range(N_CORES):
        xT = np.zeros((C, XIN), np.float32)
        xT[:, :NPIX] = x[n].reshape(NPIX, C).T
        in_maps.append({"xT": xT, **shared})
    return in_maps


def kernel(x, k1, k2, bias):
    global last_results
    if "nc" not in _cache:
        _cache["nc"] = _build_module()
    nc = _cache["nc"]
    in_maps = _host_prep(x, k1, k2, bias)
    trace = bool(int(os.environ.get("KTRACE", "0")))
    if trace:
        _ensure_axon_ntff_hook()
    res = run_bass_kernel_spmd(
        nc, in_maps, core_ids=list(range(N_CORES)), trace=trace,
    )
    last_results = res
    # Y[p, 64*ci + u] = y[128*ci + p, u]
    ys = []
    for r in res.results:
        yd = r["Y"].reshape(128, 8, COUT).transpose(1, 0, 2).reshape(1024, COUT)
        ys.append(yd[: HO * WO].reshape(HO, WO, COUT))
    return np.stack(ys, axis=0).astype(np.float32)
